# revision 48
# baseline (speedup 1.0000x reference)
"""Trainium2 Bass kernel for nn_DecoderRNN (LSTM decoder + pointer-generator).

Strategy (8 NeuronCores, SPMD, no collectives):
  - LSTM + attention replicated on every core over the full batch (the LSTM
    recurrence cost is weight-bound, independent of batch, so sharding batch
    would not reduce it; replication avoids collectives entirely).
  - The 30000-wide output matmul is vocab-sharded: core c computes output
    columns [c*3750, (c+1)*3750) and the host concatenates.
  - Pointer path is rewritten: outputs_pointer = (aw @ onehot(enc_in)) @ converter
    so the 30000-wide gather of `converter[encoder_input]` never materializes.
  - p_gen combine is folded into the matmul inputs:
        out = (H*p) @ W.T + p x linear_b + ((1-p)*S) @ converter
  - Softmax normalization is applied to exp(scores) directly (no max-subtract;
    scores are O(25) so fp32 exp is safe).

Layouts are "transposed": rows of the output (time-major index c = t*B + b)
live on the free axis; hidden/gate/vocab dims live on partitions.
"""

import os

import numpy as np

B = 32
L = 64
EMB = 256
HID = 512
VOCAB = 30000
NOBJ = 91
NCORES = 8
VL = VOCAB // NCORES  # 3750

F32 = "float32"
BF16 = "bfloat16"

_CACHE = {}
LAST_RESULT = None


# ---------------------------------------------------------------------------
# walrus CTRL-encoding legalization: hoist extra sem-waits onto same-engine NOPs
def _split_multi_waits(nc, mybir, max_waits=1):
    n_fix = 0
    for f in nc.m.functions:
        for block in f.blocks:
            insts = list(block.instructions)
            out = []
            changed = False
            for inst in insts:
                si = inst.sync_info
                waits = list(si.on_wait) if si is not None else []
                if len(waits) > max_waits:
                    extra = waits[:-max_waits]
                    keep = waits[-max_waits:]
                    chunks = [
                        extra[i : i + max_waits]
                        for i in range(0, len(extra), max_waits)
                    ]
                    for ci, chunk in enumerate(chunks):
                        nop = mybir.InstNoOp(
                            name=f"{inst.name}-waitfix-{ci}",
                            engine=inst.engine,
                            sync_info=mybir.SyncInfo(on_wait=chunk, on_update=[]),
                            bass_nofuse=True,
                        )
                        nc.register_instruction(nop)
                        out.append(nop)
                    inst.sync_info = mybir.SyncInfo(
                        on_wait=keep, on_update=list(si.on_update)
                    )
                    n_fix += 1
                    changed = True
                out.append(inst)
            if changed:
                block.instructions = out
    return n_fix


# ---------------------------------------------------------------------------
def _build(T):
    """Build the SPMD Bass program for sequence length T. Returns (nc, meta)."""
    import concourse.bass as bass
    import concourse.tile as tile
    from concourse import mybir

    dt = mybir.dt
    AF = mybir.ActivationFunctionType
    ALU = mybir.AluOpType

    R = T * B  # number of output rows
    RP = ((R + 127) // 128) * 128  # padded to full partition tiles
    MT = RP // 128  # number of 128-row output tiles
    NCH = (VL + 511) // 512  # vocab N-chunks per core

    nc = bass.Bass()

    # ---------------- DRAM I/O ----------------
    d_xt = nc.dram_tensor("xt", [2, 128, R], dt.bfloat16, kind="ExternalInput")
    d_wih = nc.dram_tensor("wih", [2, 128, 4 * HID], dt.bfloat16, kind="ExternalInput")
    d_bias = nc.dram_tensor("biaspm", [128, 16], dt.float32, kind="ExternalInput")
    d_whh = nc.dram_tensor("whh", [4, 128, 4 * HID], dt.bfloat16, kind="ExternalInput")
    d_encb = nc.dram_tensor("encb", [L, B * EMB], dt.bfloat16, kind="ExternalInput")
    d_enct = nc.dram_tensor("enct", [2, 128, B * L], dt.bfloat16, kind="ExternalInput")
    d_oh = nc.dram_tensor("oh", [L, B * NOBJ], dt.bfloat16, kind="ExternalInput")
    d_mask = nc.dram_tensor("mask01", [L, B], dt.float32, kind="ExternalInput")
    d_eye = nc.dram_tensor("eye", [128, 128], dt.bfloat16, kind="ExternalInput")
    d_awt = nc.dram_tensor("awt", [4, 128, EMB], dt.bfloat16, kind="ExternalInput")
    d_attnb = nc.dram_tensor("attnb", [128, 2], dt.float32, kind="ExternalInput")
    d_pge = nc.dram_tensor("pge", [128, 2, 32], dt.bfloat16, kind="ExternalInput")
    d_pgd = nc.dram_tensor("pgd", [128, 4, 32], dt.bfloat16, kind="ExternalInput")
    d_pb = nc.dram_tensor("pb", [32, 1], dt.float32, kind="ExternalInput")
    d_wt = nc.dram_tensor("wt", [4, 128, VL], dt.bfloat16, kind="ExternalInput")
    d_conv = nc.dram_tensor("conv", [NOBJ + 6, VL], dt.bfloat16, kind="ExternalInput")
    d_out = nc.dram_tensor("out", [R, VL], dt.bfloat16, kind="ExternalOutput")
    DBG = bool(int(os.environ.get("KDBG", "0")))
    if DBG:
        d_ebf = nc.dram_tensor("dbg_ebf", [L, R], dt.bfloat16, kind="ExternalOutput")
        d_p1 = nc.dram_tensor("dbg_p1", [32, 3 * R], dt.float32, kind="ExternalOutput")
        d_p1t = nc.dram_tensor("dbg_p1t", [32, 3 * R], dt.float32, kind="ExternalOutput")
        d_wt2 = nc.dram_tensor("dbg_wt2", [32, 2 * R], dt.float32, kind="ExternalOutput")
        d_ss2 = nc.dram_tensor("dbg_ss2", [NOBJ + 6, RP], dt.bfloat16, kind="ExternalOutput")
        d_hs = nc.dram_tensor("dbg_hs", [128, 4, RP], dt.bfloat16, kind="ExternalOutput")
        d_ht = nc.dram_tensor("dbg_ht", [128, 4, R], dt.bfloat16, kind="ExternalOutput")

    with tile.TileContext(nc) as tc:
        with (
            tc.tile_pool(name="pers", bufs=1) as pers,
            tc.tile_pool(name="arena", bufs=1) as arena,
            tc.tile_pool(name="small", bufs=2) as small,
            tc.tile_pool(name="omp", bufs=4) as omp,
            tc.tile_pool(name="psA", bufs=3, space="PSUM") as psA,
            tc.tile_pool(name="psB", bufs=1, space="PSUM") as psB,
        ):
            # ---------- persistent tiles ----------
            HT = pers.tile([128, 4, 32 * (T + 1)], dt.bfloat16, tag="HT")
            Hs = pers.tile([128, 4, RP], dt.bfloat16, tag="Hs")
            Ss2 = pers.tile([NOBJ + 6, RP], dt.bfloat16, tag="Ss")
            Ssu = pers.tile([NOBJ, RP], dt.bfloat16, tag="Ssu")
            Qsb = pers.tile([128, 2, R], dt.bfloat16, tag="Qsb")
            Ebf = pers.tile([L, R], dt.bfloat16, tag="Ebf")
            ctxsb = pers.tile([128, 2, R], dt.bfloat16, tag="ctxsb")
            enct_sb = pers.tile([128, 2, B * L], dt.bfloat16, tag="enct")
            oh_sb = pers.tile([L, B * NOBJ], dt.bfloat16, tag="oh")
            mask_sb = pers.tile([L, B], dt.float32, tag="mask")
            eye_sb = pers.tile([128, 128], dt.bfloat16, tag="eye")
            awt_sb = pers.tile([128, 4, EMB], dt.bfloat16, tag="awt")
            attnb_sb = pers.tile([128, 2], dt.float32, tag="attnb")
            pge_sb = pers.tile([128, 2, 32], dt.bfloat16, tag="pge")
            pgd_sb = pers.tile([128, 4, 32], dt.bfloat16, tag="pgd")
            pb_sb = pers.tile([32, 1], dt.float32, tag="pb")
            bias_sb = pers.tile([128, 16], dt.float32, tag="biaspm")
            conv_sb = pers.tile([NOBJ + 6, VL], dt.bfloat16, tag="conv")
            ones_f = pers.tile([1, 128], dt.float32, tag="ones_f")
            ones64 = pers.tile([L, 1], dt.bfloat16, tag="ones64")
            ones_b64 = pers.tile([L, 32], dt.bfloat16, tag="ones_b64")
            ones_bf = pers.tile([64, 128], dt.bfloat16, tag="ones_bf")

            nc.vector.memset(ones_f[:], 1.0)
            nc.vector.memset(ones64[:], 1.0)
            nc.vector.memset(ones_bf[:], 1.0)
            nc.vector.memset(ones_b64[:], 1.0)

            # ---------- phase-0 DMAs (sync/HWDGE) ----------
            xt_sb = arena.tile([128, 2, R], dt.bfloat16, tag="slotD")
            wih_sb = arena.tile([128, 2, 4 * HID], dt.bfloat16, tag="slotC")
            Gx = arena.tile([128, 16, R], dt.bfloat16, tag="slotA")
            whh_sb = arena.tile([128, 4, 4 * HID], dt.bfloat16, tag="slotB")

            # spread input loads over three DMA queues so the Gx inputs
            # (sync queue) are not stuck behind the big attention loads
            # critical-path loads only (Gx then LSTM); everything needed for
            # the attention/vocab phases is issued after the Gx matmuls so it
            # doesn't steal DMA bandwidth from the startup
            # consumption-ordered pieces so the first Gx matmuls start as
            # soon as their slices land, not when whole tensors finish
            nc.sync.dma_start(bias_sb[:], d_bias[:])
            nc.sync.dma_start(xt_sb[:, 0, 0:512], d_xt[0][:, 0:512])
            nc.scalar.dma_start(xt_sb[:, 1, 0:512], d_xt[1][:, 0:512])
            nc.sync.dma_start(wih_sb[:, 0, 0:1024], d_wih[0][:, 0:1024])
            nc.scalar.dma_start(wih_sb[:, 1, 0:1024], d_wih[1][:, 0:1024])
            nc.sync.dma_start(eye_sb[:], d_eye[:])
            nc.sync.dma_start(wih_sb[:, 0, 1024:], d_wih[0][:, 1024:])
            nc.scalar.dma_start(wih_sb[:, 1, 1024:], d_wih[1][:, 1024:])
            nc.sync.dma_start(xt_sb[:, 0, 512:], d_xt[0][:, 512:])
            nc.scalar.dma_start(xt_sb[:, 1, 512:], d_xt[1][:, 512:])
            nc.scalar.dma_start(whh_sb[:], d_whh[:].rearrange("k p m -> p k m"))

            # ---------- Gx = W_ih' @ X^T  (+bias on copy-out), bf16 ----------
            # Gx layout [128, 16 gate-tiles, R]; column c = t*B + b.
            gx_gate_mm = None
            for n0 in range(0, R, 512):
                nn = min(512, R - n0)
                for m in range(16):
                    ps = psA.tile([128, 512], dt.float32, tag="psA", bufs=4)
                    for k in range(2):
                        _mm = nc.tensor.matmul(
                            ps[:, :nn],
                            wih_sb[:, k, m * 128 : (m + 1) * 128],
                            xt_sb[:, k, n0 : n0 + nn],
                            start=(k == 0),
                            stop=(k == 1),
                        )
                        if gx_gate_mm is None:
                            gx_gate_mm = _mm
                        gx_last_mm = _mm
                    if (m + n0 // 512) % 2 == 0:
                        nc.scalar.activation(
                            Gx[:, m, n0 : n0 + nn], ps[:, :nn],
                            AF.Identity, bias=bias_sb[:, m : m + 1],
                        )
                    else:
                        nc.vector.tensor_scalar(
                            out=Gx[:, m, n0 : n0 + nn],
                            in0=ps[:, :nn],
                            scalar1=bias_sb[:, m : m + 1],
                            scalar2=None,
                            op0=ALU.add,
                        )

            # attention/vocab-phase loads (needed ~150us later); explicitly
            # held back behind the first Gx matmul so they don't steal DMA
            # bandwidth from the startup-critical loads
            import bass_rust as _br

            late = []
            late.append(nc.scalar.dma_start(enct_sb[:], d_enct[:].rearrange("k p n -> p k n")))
            late.append(nc.gpsimd.dma_start(oh_sb[:], d_oh[:]))
            late.append(nc.gpsimd.dma_start(mask_sb[:], d_mask[:]))
            late.append(nc.gpsimd.dma_start(awt_sb[:], d_awt[:].rearrange("k p m -> p k m")))
            late.append(nc.gpsimd.dma_start(attnb_sb[:], d_attnb[:]))
            late.append(nc.gpsimd.dma_start(pge_sb[:], d_pge[:]))
            late.append(nc.gpsimd.dma_start(pgd_sb[:], d_pgd[:]))
            late.append(nc.gpsimd.dma_start(pb_sb[:], d_pb[:]))
            late.append(nc.gpsimd.dma_start(conv_sb[:], d_conv[:]))
            encb_sb = pers.tile([L, B * EMB], dt.bfloat16, tag="encb")
            late.append(nc.gpsimd.dma_start(encb_sb[:], d_encb[:]))
            Wt_sb = pers.tile([128, 4, VL], dt.bfloat16, tag="Wt")
            late.append(nc.gpsimd.dma_start(Wt_sb[:], d_wt[:].rearrange("k p v -> p k v")))
            for dma in late:
                _br.add_dep_helper(
                    dma.ins, gx_last_mm.ins, True,
                    "hold noncritical DMA behind the Gx phase",
                )

            # ---------- LSTM over T steps ----------
            # HT column slots: slot 0 = h_{-1} = 0; step t writes slot t+1.
            nc.vector.memset(HT[:, :, 0:32], 0.0)

            # Gate tiles are host-permuted into two half-banks:
            #   bank A (m 0..7)  = [i0 i1 f0 f1 o0 o1 g0 g1]  -> h quarters 0,1
            #   bank B (m 8..15) = [i2 i3 f2 f3 o2 o3 g2 g3]  -> h quarters 2,3
            # Each bank gets its own PSUM tile, preloaded with Gx_t via an
            # eye-matmul two steps ahead (PE-idle time), so the tail of half A
            # can start after only the A-half of the W_hh matmuls.
            psl = [None] * (T + 2)

            def eye_preload(t):
                if t >= T:
                    return
                pa = psA.tile([128, 256], dt.float32, tag="psLa", bufs=2)
                pb = psA.tile([128, 256], dt.float32, tag="psLb", bufs=2)
                nc.tensor.matmul(
                    pa[:, :], eye_sb[:],
                    Gx[:, 0:8, t * 32 : (t + 1) * 32],
                    start=True, stop=True,
                )
                nc.tensor.matmul(
                    pb[:, :], eye_sb[:],
                    Gx[:, 8:16, t * 32 : (t + 1) * 32],
                    start=True, stop=True,
                )
                psl[t] = (pa, pb)

            eye_preload(0)

            cprev = None
            act_prev = None  # forced ACT-queue chain (scheduler reorders otherwise)

            def act_chain(ins):
                nonlocal act_prev
                if act_prev is not None:
                    _br.add_dep_helper(ins.ins, act_prev.ins, True, "ACT order")
                act_prev = ins
                return ins

            for t in range(T):
                ht_prev = HT[:, :, t * 32 : (t + 1) * 32]
                pa, pb = psl[t]
                # += W_hh @ h_{t-1}; A-half first so its tail starts earliest.
                for half, ps in ((0, pa), (1, pb)):
                    for k in range(4):
                        for m in range(8):
                            nc.tensor.matmul(
                                ps[:, m * 32 : (m + 1) * 32],
                                whh_sb[:, k, (8 * half + m) * 128 : (8 * half + m + 1) * 128],
                                ht_prev[:, k, :],
                                start=False,
                                stop=(k == 3),
                                skip_group_check=True,
                            )
                eye_preload(t + 1)
                cnew = small.tile([128, 4, 32], dt.float32, tag="c_t")
                sigs = []
                tgs = []
                # ACT emission order: sigA, tgA, sigB, tcA, tgB, tcB — sigB
                # fills the ACT gap while DVE computes c half A.
                for hx, ps in ((0, pa), (1, pb)):
                    # bank layout [i i f f o o g g] x 32 cols
                    sig = small.tile([128, 3, 2, 32], dt.float32, tag=f"sig{hx}")
                    act_chain(nc.scalar.activation(
                        sig[:],
                        ps[:, 0:192].rearrange("p (g u x) -> p g u x", g=3, u=2),
                        AF.Sigmoid,
                    ))
                    sigs.append(sig)
                    tg = small.tile([128, 2, 32], dt.float32, tag=f"tg{hx}")
                    tgs.append(tg)
                    if hx == 0:
                        act_chain(nc.scalar.activation(
                            tg[:], ps[:, 192:256].rearrange("p (u x) -> p u x", u=2),
                            AF.Tanh,
                        ))
                for hx in (0, 1):
                    sig, tg = sigs[hx], tgs[hx]
                    if hx == 1:
                        act_chain(nc.scalar.activation(
                            tg[:], pb[:, 192:256].rearrange("p (u x) -> p u x", u=2),
                            AF.Tanh,
                        ))
                    if cprev is not None:
                        b_t = small.tile([128, 2, 32], dt.float32, tag=f"b_t{hx}")
                        nc.vector.tensor_mul(
                            b_t[:], sig[:, 1], cprev[:, 2 * hx : 2 * hx + 2]
                        )
                    a_t = small.tile([128, 2, 32], dt.float32, tag=f"a_t{hx}")
                    nc.vector.tensor_mul(a_t[:], sig[:, 0], tg[:])
                    if cprev is None:
                        nc.vector.tensor_copy(cnew[:, 2 * hx : 2 * hx + 2], a_t[:])
                    else:
                        nc.vector.tensor_add(
                            cnew[:, 2 * hx : 2 * hx + 2], a_t[:], b_t[:]
                        )
                    tc_t = small.tile([128, 2, 32], dt.float32, tag=f"tc_t{hx}")
                    act_chain(nc.scalar.activation(
                        tc_t[:], cnew[:, 2 * hx : 2 * hx + 2], AF.Tanh
                    ))
                    # per-quarter h writes (exact-match deps: the next step's
                    # k-pass waits only on its own quarter)
                    for u in (0, 1):
                        nc.vector.tensor_mul(
                            HT[
                                :,
                                2 * hx + u : 2 * hx + u + 1,
                                (t + 1) * 32 : (t + 2) * 32,
                            ],
                            sig[:, 2, u : u + 1],
                            tc_t[:, u : u + 1],
                        )
                cprev = cnew

            HTv = HT[:, :, 32 : 32 + R]  # h_1..h_T columns, time-major

            # ---------- attention (chunked psum, batch-major) ----------
            # scratch reuses arena slots that die with the LSTM
            P1 = arena.tile([32, 3 * R], dt.float32, tag="slotA")
            P1t = arena.tile([32, 3 * R], dt.float32, tag="slotB")
            Vt = arena.tile([32, 2 * R], dt.float32, tag="slotC")
            Wt2 = arena.tile([32, 2 * R], dt.float32, tag="slotD")
            nc.vector.memset(Vt[:], 0.0)
            # Everything after the LSTM works in batch-major columns
            # (c = b*T + t); time-major consumers read via strided views.
            # Q^T [256, R] time-major (scores read per-batch slices of it)
            for m in range(2):
                for n0 in range(0, R, 512):
                    nn = min(512, R - n0)
                    qp = psA.tile([128, 512], dt.float32, tag="psA", bufs=4)
                    for k in range(4):
                        nc.tensor.matmul(
                            qp[:, :nn],
                            awt_sb[:, k, m * 128 : (m + 1) * 128],
                            HTv[:, k, :][:, n0 : n0 + nn],
                            start=(k == 0),
                            stop=(k == 3),
                        )
                    if m == 0:
                        nc.vector.tensor_scalar(
                            out=Qsb[:, m, n0 : n0 + nn], in0=qp[:, :nn],
                            scalar1=attnb_sb[:, m : m + 1], scalar2=None,
                            op0=ALU.add,
                        )
                    else:
                        nc.scalar.activation(
                            Qsb[:, m, n0 : n0 + nn], qp[:, :nn],
                            AF.Identity, bias=attnb_sb[:, m : m + 1],
                        )

            # scores/E chunks [64, 512] batch-major; exp+mask fused per chunk
            for n0 in range(0, R, 512):
                nn = min(512, R - n0)
                nb = nn // T
                b0 = n0 // T
                sct = psA.tile([128, 512], dt.float32, tag="psA", bufs=4)
                for bb in range(nb):
                    b = b0 + bb
                    for k in range(2):
                        nc.tensor.matmul(
                            sct[0:L, bb * T : (bb + 1) * T],
                            enct_sb[:, k, b * L : (b + 1) * L],
                            Qsb[:, k, :].rearrange("p (t bb) -> p bb t", bb=B)[:, b, :],
                            start=(k == 0),
                            stop=(k == 1),
                        )
                nc.scalar.activation(Ebf[:, n0 : n0 + nn], sct[0:L, :nn], AF.Exp)
                mb = mask_sb[:, :]
                nc.vector.tensor_mul(
                    Ebf[:, n0 : n0 + nn].rearrange("p (bb t) -> p bb t", t=T),
                    Ebf[:, n0 : n0 + nn].rearrange("p (bb t) -> p bb t", t=T),
                    bass.AP(tensor=mb.tensor, offset=mb.offset + b0,
                            ap=[list(mb.ap[0]), [1, nb], [0, T]]),
                )

            # ctx_un^T [256, R] bf16 batch-major
            for m in range(2):
                for n0 in range(0, R, 512):
                    nn = min(512, R - n0)
                    nb = nn // T
                    b0 = n0 // T
                    cp = psA.tile([128, 512], dt.float32, tag="psA", bufs=4)
                    for bb in range(nb):
                        b = b0 + bb
                        nc.tensor.matmul(
                            cp[:, bb * T : (bb + 1) * T],
                            encb_sb[:, b * EMB + m * 128 : b * EMB + (m + 1) * 128],
                            Ebf[:, b * T : (b + 1) * T],
                        )
                    if m == 0:
                        nc.scalar.copy(ctxsb[:, m, n0 : n0 + nn], cp[:, :nn])
                    else:
                        nc.vector.tensor_copy(ctxsb[:, m, n0 : n0 + nn], cp[:, :nn])

            # cs/pp/ph as 32-row-replicated psum blocks -> P1 [96, R] sbuf
            for n0 in range(0, R, 512):
                nn = min(512, R - n0)
                csp = psA.tile([128, 512], dt.float32, tag="psA", bufs=4)
                nc.tensor.matmul(csp[0:32, :nn], ones_b64[:], Ebf[:, n0 : n0 + nn])
                nc.scalar.copy(P1[:, n0 : n0 + nn], csp[0:32, :nn])
                ppp = psA.tile([128, 512], dt.float32, tag="psA", bufs=4)
                for k in range(2):
                    nc.tensor.matmul(
                        ppp[0:32, :nn], pge_sb[:, k], ctxsb[:, k, n0 : n0 + nn],
                        start=(k == 0), stop=(k == 1),
                    )
                nc.vector.tensor_copy(P1[:, R + n0 : R + n0 + nn], ppp[0:32, :nn])
                php = psA.tile([128, 512], dt.float32, tag="psA", bufs=4)
                for k in range(4):
                    nc.tensor.matmul(
                        php[0:32, :nn], pgd_sb[:, k], HTv[:, k, :][:, n0 : n0 + nn],
                        start=(k == 0), stop=(k == 3),
                    )
                nc.scalar.copy(P1[:, 2 * R + n0 : 2 * R + n0 + nn], php[0:32, :nn])

            # S_un^T [91, R] bf16 batch-major
            for n0 in range(0, R, 512):
                nn = min(512, R - n0)
                nb = nn // T
                b0 = n0 // T
                spt = psA.tile([128, 512], dt.float32, tag="psA", bufs=4)
                for bb in range(nb):
                    b = b0 + bb
                    nc.tensor.matmul(
                        spt[0:NOBJ, bb * T : (bb + 1) * T],
                        oh_sb[:, b * NOBJ : (b + 1) * NOBJ],
                        Ebf[:, b * T : (b + 1) * T],
                    )
                nb2 = nn // T
                b02 = n0 // T
                nc.vector.tensor_copy(
                    Ssu[:, 0:R].rearrange("p (t bb) -> p t bb", bb=B)[:, :, b02 : b02 + nb2],
                    spt[0:NOBJ, :nn].rearrange("p (bb t) -> p t bb", t=T),
                )

            # p_gen math on 32x32 stream-transposed data (no [1,R] lane ops)
            nc.vector.transpose(P1t[:], P1[:])
            csv = P1t[:, 0:R].rearrange("p (j f) -> p j f", f=32)[:, :, 0]
            ppv = P1t[:, R : 2 * R].rearrange("p (j f) -> p j f", f=32)[:, :, 0]
            phv = P1t[:, 2 * R : 3 * R].rearrange("p (j f) -> p j f", f=32)[:, :, 0]
            rv = small.tile([32, 32], dt.float32, tag="rv32")
            nc.vector.reciprocal(rv[:], csv)
            en32 = small.tile([32, 32], dt.float32, tag="en32")
            nc.vector.tensor_mul(en32[:], rv[:], ppv)
            # phv is time-major-folded; transpose its 32x32 to match (b=j,t=p)
            phc = small.tile([32, 32], dt.float32, tag="phc")
            nc.vector.tensor_copy(phc[:], phv)
            ph32 = small.tile([32, 32], dt.float32, tag="ph32")
            nc.vector.transpose(ph32[:], phc[:])
            den32 = small.tile([32, 32], dt.float32, tag="den32")
            nc.vector.tensor_add(den32[:], en32[:], ph32[:])
            p32 = small.tile([32, 32], dt.float32, tag="p32")
            nc.scalar.activation(p32[:], den32[:], AF.Sigmoid, bias=pb_sb[:, 0:1])
            q32 = small.tile([32, 32], dt.float32, tag="q32")
            nc.vector.tensor_scalar(
                out=q32[:], in0=p32[:], scalar1=-1.0, scalar2=1.0,
                op0=ALU.mult, op1=ALU.add,
            )
            s32 = small.tile([32, 32], dt.float32, tag="s32")
            nc.vector.tensor_mul(s32[:], rv[:], q32[:])
            p32T = small.tile([32, 32], dt.float32, tag="p32T")
            nc.vector.transpose(p32T[:], p32[:])
            s32T = small.tile([32, 32], dt.float32, tag="s32T")
            nc.vector.transpose(s32T[:], s32[:])
            nc.vector.tensor_copy(
                Vt[:, 0:R].rearrange("p (j f) -> p j f", f=32)[:, :, 0], p32T[:]
            )
            nc.vector.tensor_copy(
                Vt[:, R : 2 * R].rearrange("p (j f) -> p j f", f=32)[:, :, 0], s32T[:]
            )
            nc.vector.transpose(Wt2[:], Vt[:])
            p_row = Wt2[0:1, 0:R]           # [1, R] TIME-major
            sscl_row = Wt2[0:1, R : 2 * R]  # [1, R] TIME-major

            # Hs = H * bcast(p)  (time-major bf16) ; Ss2 row 96 = p
            nc.vector.memset(Ss2[64:96, :], 0.0)
            for n0 in range(0, R, 512):
                nn = min(512, R - n0)
                pbt = psA.tile([128, 512], dt.float32, tag="psA", bufs=4)
                nc.tensor.matmul(pbt[:, :nn], ones_f[:], p_row[:, n0 : n0 + nn])
                for k in range(4):
                    nc.vector.tensor_mul(
                        Hs[:, k, n0 : n0 + nn], HTv[:, k, :][:, n0 : n0 + nn],
                        pbt[:, :nn],
                    )
                qbt = psA.tile([128, 512], dt.float32, tag="psA", bufs=4)
                nc.tensor.matmul(
                    qbt[0:NOBJ, :nn], ones_f[:, 0:NOBJ], sscl_row[:, n0 : n0 + nn]
                )
                nc.vector.tensor_mul(
                    Ss2[0:NOBJ, n0 : n0 + nn], Ssu[:, n0 : n0 + nn], qbt[0:NOBJ, :nn]
                )
            nc.scalar.copy(Ss2[96:97, :], p_row)

            if DBG:
                nc.sync.dma_start(d_ebf[:], Ebf[:])
                nc.sync.dma_start(d_p1[:], P1[:])
                nc.sync.dma_start(d_p1t[:], P1t[:])
                nc.sync.dma_start(d_wt2[:], Wt2[:])
                nc.sync.dma_start(d_ss2[:], Ss2[:])
                nc.sync.dma_start(d_hs[:], Hs[:])
                nc.sync.dma_start(d_ht[:], HTv[:, :, :])

            # ---------- vocab matmul, vocab-sharded ----------
            # lhsT reads Hs/Ss2 (batch-major) through time-major strided views
            for m in range(MT):
                rlo = m * 128
                for n0 in range(0, VL, 512):
                    nn = min(512, VL - n0)
                    ps = psA.tile([128, 512], dt.float32, tag="psA", bufs=4)
                    for k in range(4):
                        nc.tensor.matmul(
                            ps[:, :nn],
                            Hs[:, k, rlo : rlo + 128],
                            Wt_sb[:, k, n0 : n0 + nn],
                            start=(k == 0),
                            stop=False,
                        )
                    # pointer logits + p*linear_b in one K=97 matmul
                    nc.tensor.matmul(
                        ps[:, :nn],
                        Ss2[:, rlo : rlo + 128],
                        conv_sb[:, n0 : n0 + nn],
                        start=False,
                        stop=True,
                    )
                    om = omp.tile([128, 512], dt.bfloat16, tag="om")
                    if (m + n0 // 512) % 2 == 0:
                        nc.scalar.copy(om[:, :nn], ps[:, :nn])
                        nc.sync.dma_start(d_out[rlo : rlo + 128, n0 : n0 + nn], om[:, :nn])
                    else:
                        nc.vector.tensor_copy(om[:, :nn], ps[:, :nn])
                        nc.scalar.dma_start(d_out[rlo : rlo + 128, n0 : n0 + nn], om[:, :nn])

    n_fix = _split_multi_waits(nc, mybir, max_waits=1)
    nc.finalize()
    return nc


def _prep_inputs(features, captions, lengths, encoder_input, encoder_output,
                 embed_W, W_ih, W_hh, b_ih, b_hh, linear_W, linear_b,
                 attn_W, attn_b, pge_W, pge_b, pgd_W, pgd_b, converter):
    """Host-side sharding/layout prep. Returns per-core in_maps and T."""
    import ml_dtypes

    bf16 = ml_dtypes.bfloat16
    f32 = np.float32

    features = np.asarray(features, f32)
    captions = np.asarray(captions)
    encoder_input = np.asarray(encoder_input)
    encoder_output = np.asarray(encoder_output, f32)
    embed_W = np.asarray(embed_W, f32)
    W_ih = np.asarray(W_ih, f32)
    W_hh = np.asarray(W_hh, f32)
    b_ih = np.asarray(b_ih, f32)
    b_hh = np.asarray(b_hh, f32)
    linear_W = np.asarray(linear_W, f32)
    linear_b = np.asarray(linear_b, f32)
    attn_W = np.asarray(attn_W, f32)
    attn_b = np.asarray(attn_b, f32)
    pge_W = np.asarray(pge_W, f32)
    pge_b = np.asarray(pge_b, f32)
    pgd_W = np.asarray(pgd_W, f32)
    pgd_b = np.asarray(pgd_b, f32)
    converter = np.asarray(converter, f32)

    T = int(lengths)
    R = T * B

    # x sequence: t=0 -> features, t>=1 -> embed_W[captions[:, t-1]]
    emb = np.empty((B, T, EMB), f32)
    emb[:, 0, :] = features
    if T > 1:
        emb[:, 1:, :] = embed_W[captions[:, : T - 1]]
    # XT [EMB, R], column c = t*B + b
    XT = np.ascontiguousarray(emb.transpose(2, 1, 0).reshape(EMB, R))
    xt = XT.reshape(2, 128, R).astype(bf16)

    # gate permutation into two half-banks (torch chunk order: i=0-3, f=4-7,
    # g=8-11, o=12-15): A = [i0 i1 f0 f1 o0 o1 g0 g1], B = same second halves
    chunk_perm = [0, 1, 4, 5, 12, 13, 8, 9, 2, 3, 6, 7, 14, 15, 10, 11]
    perm = np.concatenate([np.arange(c * 128, (c + 1) * 128) for c in chunk_perm])
    wih = np.ascontiguousarray(W_ih[perm].T).reshape(2, 128, 4 * HID).astype(bf16)
    whh = np.ascontiguousarray(W_hh[perm].T).reshape(4, 128, 4 * HID).astype(bf16)
    biasv = (b_ih + b_hh)[perm].astype(f32)
    biaspm = np.ascontiguousarray(biasv.reshape(16, 128).T)

    encb = np.ascontiguousarray(
        encoder_output.transpose(1, 0, 2).reshape(L, B * EMB)
    ).astype(bf16)
    enct = np.ascontiguousarray(
        encoder_output.transpose(2, 0, 1).reshape(2, 128, B * L)
    ).astype(bf16)
    oh = np.ascontiguousarray(
        np.eye(NOBJ, dtype=f32)[encoder_input].transpose(1, 0, 2).reshape(L, B * NOBJ)
    ).astype(bf16)
    mask01 = np.ascontiguousarray((encoder_input.T != 0).astype(f32))  # [L, B]
    eye = np.eye(128, dtype=f32).astype(bf16)

    awt = np.ascontiguousarray(attn_W.T).reshape(4, 128, EMB).astype(bf16)
    attnb = np.ascontiguousarray(attn_b.reshape(2, 128).T).astype(f32)
    pge = np.repeat(
        np.ascontiguousarray(pge_W.reshape(EMB).reshape(2, 128).T)[:, :, None], 32, axis=2
    ).astype(bf16)
    pgd = np.repeat(
        np.ascontiguousarray(pgd_W.reshape(HID).reshape(4, 128).T)[:, :, None], 32, axis=2
    ).astype(bf16)
    pb = np.full((32, 1), float(pge_b.reshape(-1)[0] + pgd_b.reshape(-1)[0]), f32)

    common = dict(
        xt=xt, wih=wih, biaspm=biaspm, whh=whh, encb=encb, enct=enct,
        oh=oh, mask01=mask01, eye=eye, awt=awt, attnb=attnb, pge=pge,
        pgd=pgd, pb=pb,
    )

    in_maps = []
    for c in range(NCORES):
        v0, v1 = c * VL, (c + 1) * VL
        wt = np.ascontiguousarray(linear_W[v0:v1].T).reshape(4, 128, VL).astype(bf16)
        # converter slice with linear_b appended as row 91 (paired with the
        # p_gen row of Ss2 on device)
        conv = np.ascontiguousarray(
            np.concatenate(
                [converter[:, v0:v1], np.zeros((5, VL), f32),
                 linear_b[v0:v1][None, :]], axis=0)
        ).astype(bf16)
        m = dict(common)
        m.update(wt=wt, conv=conv)
        in_maps.append(m)
    return in_maps, T, R


def kernel(**inputs):
    global LAST_RESULT
    from concourse.bass_utils import run_bass_kernel_spmd

    in_maps, T, R = _prep_inputs(**inputs)
    if T not in _CACHE:
        _CACHE[T] = _build(T)
    nc = _CACHE[T]

    res = run_bass_kernel_spmd(nc, in_maps, core_ids=list(range(NCORES)))
    LAST_RESULT = res
    out = np.concatenate([res.results[c]["out"] for c in range(NCORES)], axis=1)
    return out.astype(np.float32)



# revision 50
# speedup vs baseline: 1.0211x; 1.0211x over previous
"""Trainium2 Bass kernel for nn_DecoderRNN (LSTM decoder + pointer-generator).

Strategy (8 NeuronCores, SPMD, no collectives):
  - LSTM + attention replicated on every core over the full batch (the LSTM
    recurrence cost is weight-bound, independent of batch, so sharding batch
    would not reduce it; replication avoids collectives entirely).
  - The 30000-wide output matmul is vocab-sharded: core c computes output
    columns [c*3750, (c+1)*3750) and the host concatenates.
  - Pointer path is rewritten: outputs_pointer = (aw @ onehot(enc_in)) @ converter
    so the 30000-wide gather of `converter[encoder_input]` never materializes.
  - p_gen combine is folded into the matmul inputs:
        out = (H*p) @ W.T + p x linear_b + ((1-p)*S) @ converter
  - Softmax normalization is applied to exp(scores) directly (no max-subtract;
    scores are O(25) so fp32 exp is safe).

Layouts are "transposed": rows of the output (time-major index c = t*B + b)
live on the free axis; hidden/gate/vocab dims live on partitions.
"""

import os

import numpy as np

B = 32
L = 64
EMB = 256
HID = 512
VOCAB = 30000
NOBJ = 91
NCORES = 8
VL = VOCAB // NCORES  # 3750

F32 = "float32"
BF16 = "bfloat16"

_CACHE = {}
LAST_RESULT = None


# ---------------------------------------------------------------------------
# walrus CTRL-encoding legalization: hoist extra sem-waits onto same-engine NOPs
def _split_multi_waits(nc, mybir, max_waits=1):
    n_fix = 0
    for f in nc.m.functions:
        for block in f.blocks:
            insts = list(block.instructions)
            out = []
            changed = False
            for inst in insts:
                si = inst.sync_info
                waits = list(si.on_wait) if si is not None else []
                if len(waits) > max_waits:
                    extra = waits[:-max_waits]
                    keep = waits[-max_waits:]
                    chunks = [
                        extra[i : i + max_waits]
                        for i in range(0, len(extra), max_waits)
                    ]
                    for ci, chunk in enumerate(chunks):
                        nop = mybir.InstNoOp(
                            name=f"{inst.name}-waitfix-{ci}",
                            engine=inst.engine,
                            sync_info=mybir.SyncInfo(on_wait=chunk, on_update=[]),
                            bass_nofuse=True,
                        )
                        nc.register_instruction(nop)
                        out.append(nop)
                    inst.sync_info = mybir.SyncInfo(
                        on_wait=keep, on_update=list(si.on_update)
                    )
                    n_fix += 1
                    changed = True
                out.append(inst)
            if changed:
                block.instructions = out
    return n_fix


# ---------------------------------------------------------------------------
def _build(T):
    """Build the SPMD Bass program for sequence length T. Returns (nc, meta)."""
    import concourse.bass as bass
    import concourse.tile as tile
    from concourse import mybir

    dt = mybir.dt
    AF = mybir.ActivationFunctionType
    ALU = mybir.AluOpType

    R = T * B  # number of output rows
    RP = ((R + 127) // 128) * 128  # padded to full partition tiles
    MT = RP // 128  # number of 128-row output tiles
    NCH = (VL + 511) // 512  # vocab N-chunks per core

    nc = bass.Bass()

    # ---------------- DRAM I/O ----------------
    d_xt = nc.dram_tensor("xt", [2, 128, R], dt.bfloat16, kind="ExternalInput")
    d_wih = nc.dram_tensor("wih", [2, 128, 4 * HID], dt.bfloat16, kind="ExternalInput")
    d_bias = nc.dram_tensor("biaspm", [128, 16], dt.float32, kind="ExternalInput")
    d_whh = nc.dram_tensor("whh", [4, 128, 4 * HID], dt.bfloat16, kind="ExternalInput")
    d_encb = nc.dram_tensor("encb", [L, B * EMB], dt.bfloat16, kind="ExternalInput")
    d_enct = nc.dram_tensor("enct", [2, 128, B * L], dt.bfloat16, kind="ExternalInput")
    d_oh = nc.dram_tensor("oh", [L, B * NOBJ], dt.bfloat16, kind="ExternalInput")
    d_mask = nc.dram_tensor("mask01", [L, B], dt.float32, kind="ExternalInput")
    d_eye = nc.dram_tensor("eye", [128, 128], dt.bfloat16, kind="ExternalInput")
    d_awt = nc.dram_tensor("awt", [4, 128, EMB], dt.bfloat16, kind="ExternalInput")
    d_attnb = nc.dram_tensor("attnb", [128, 2], dt.float32, kind="ExternalInput")
    d_pge = nc.dram_tensor("pge", [128, 2, 32], dt.bfloat16, kind="ExternalInput")
    d_pgd = nc.dram_tensor("pgd", [128, 4, 32], dt.bfloat16, kind="ExternalInput")
    d_pb = nc.dram_tensor("pb", [32, 1], dt.float32, kind="ExternalInput")
    d_wt = nc.dram_tensor("wt", [4, 128, VL], dt.bfloat16, kind="ExternalInput")
    d_conv = nc.dram_tensor("conv", [NOBJ + 6, VL], dt.bfloat16, kind="ExternalInput")
    d_out = nc.dram_tensor("out", [R, VL], dt.bfloat16, kind="ExternalOutput")
    DBG = bool(int(os.environ.get("KDBG", "0")))
    if DBG:
        d_ebf = nc.dram_tensor("dbg_ebf", [L, R], dt.bfloat16, kind="ExternalOutput")
        d_p1 = nc.dram_tensor("dbg_p1", [32, 3 * R], dt.float32, kind="ExternalOutput")
        d_p1t = nc.dram_tensor("dbg_p1t", [32, 3 * R], dt.float32, kind="ExternalOutput")
        d_wt2 = nc.dram_tensor("dbg_wt2", [32, 2 * R], dt.float32, kind="ExternalOutput")
        d_ss2 = nc.dram_tensor("dbg_ss2", [NOBJ + 6, RP], dt.bfloat16, kind="ExternalOutput")
        d_hs = nc.dram_tensor("dbg_hs", [128, 4, RP], dt.bfloat16, kind="ExternalOutput")
        d_ht = nc.dram_tensor("dbg_ht", [128, 4, R], dt.bfloat16, kind="ExternalOutput")

    with tile.TileContext(nc) as tc:
        with (
            tc.tile_pool(name="pers", bufs=1) as pers,
            tc.tile_pool(name="arena", bufs=1) as arena,
            tc.tile_pool(name="small", bufs=2) as small,
            tc.tile_pool(name="omp", bufs=4) as omp,
            tc.tile_pool(name="psA", bufs=3, space="PSUM") as psA,
            tc.tile_pool(name="psB", bufs=1, space="PSUM") as psB,
        ):
            # ---------- persistent tiles ----------
            HT = pers.tile([128, 4, 32 * (T + 1)], dt.bfloat16, tag="HT")
            Hs = pers.tile([128, 4, RP], dt.bfloat16, tag="Hs")
            Ss2 = pers.tile([NOBJ + 6, RP], dt.bfloat16, tag="Ss")
            Ssu = pers.tile([NOBJ, RP], dt.bfloat16, tag="Ssu")
            Qsb = pers.tile([128, 2, R], dt.bfloat16, tag="Qsb")
            Ebf = pers.tile([L, R], dt.bfloat16, tag="Ebf")
            ctxsb = pers.tile([128, 2, R], dt.bfloat16, tag="ctxsb")
            enct_sb = pers.tile([128, 2, B * L], dt.bfloat16, tag="enct")
            oh_sb = pers.tile([L, B * NOBJ], dt.bfloat16, tag="oh")
            mask_sb = pers.tile([L, B], dt.float32, tag="mask")
            eye_sb = pers.tile([128, 128], dt.bfloat16, tag="eye")
            awt_sb = pers.tile([128, 4, EMB], dt.bfloat16, tag="awt")
            attnb_sb = pers.tile([128, 2], dt.float32, tag="attnb")
            pge_sb = pers.tile([128, 2, 32], dt.bfloat16, tag="pge")
            pgd_sb = pers.tile([128, 4, 32], dt.bfloat16, tag="pgd")
            pb_sb = pers.tile([32, 1], dt.float32, tag="pb")
            bias_sb = pers.tile([128, 16], dt.float32, tag="biaspm")
            conv_sb = pers.tile([NOBJ + 6, VL], dt.bfloat16, tag="conv")
            ones_f = pers.tile([1, 128], dt.float32, tag="ones_f")
            ones64 = pers.tile([L, 1], dt.bfloat16, tag="ones64")
            ones_b64 = pers.tile([L, 32], dt.bfloat16, tag="ones_b64")
            ones_bf = pers.tile([64, 128], dt.bfloat16, tag="ones_bf")

            nc.vector.memset(ones_f[:], 1.0)
            nc.vector.memset(ones64[:], 1.0)
            nc.vector.memset(ones_bf[:], 1.0)
            nc.vector.memset(ones_b64[:], 1.0)

            # ---------- phase-0 DMAs (sync/HWDGE) ----------
            xt_sb = arena.tile([128, 2, R], dt.bfloat16, tag="slotD")
            wih_sb = arena.tile([128, 2, 4 * HID], dt.bfloat16, tag="slotC")
            Gx = arena.tile([128, 16, R], dt.bfloat16, tag="slotA")
            whh_sb = arena.tile([128, 4, 4 * HID], dt.bfloat16, tag="slotB")

            # spread input loads over three DMA queues so the Gx inputs
            # (sync queue) are not stuck behind the big attention loads
            # critical-path loads only (Gx then LSTM); everything needed for
            # the attention/vocab phases is issued after the Gx matmuls so it
            # doesn't steal DMA bandwidth from the startup
            nc.sync.dma_start(xt_sb[:, 0], d_xt[0])
            nc.scalar.dma_start(xt_sb[:, 1], d_xt[1])
            nc.sync.dma_start(wih_sb[:, 0], d_wih[0])
            nc.scalar.dma_start(wih_sb[:, 1], d_wih[1])
            nc.sync.dma_start(bias_sb[:], d_bias[:])
            nc.sync.dma_start(eye_sb[:], d_eye[:])
            nc.scalar.dma_start(whh_sb[:], d_whh[:].rearrange("k p m -> p k m"))

            # ---------- Gx = W_ih' @ X^T  (+bias on copy-out), bf16 ----------
            # Gx layout [128, 16 gate-tiles, R]; column c = t*B + b.
            gx_gate_mm = None
            for n0 in range(0, R, 512):
                nn = min(512, R - n0)
                for m in range(16):
                    ps = psA.tile([128, 512], dt.float32, tag="psA", bufs=4)
                    for k in range(2):
                        _mm = nc.tensor.matmul(
                            ps[:, :nn],
                            wih_sb[:, k, m * 128 : (m + 1) * 128],
                            xt_sb[:, k, n0 : n0 + nn],
                            start=(k == 0),
                            stop=(k == 1),
                        )
                        if gx_gate_mm is None:
                            gx_gate_mm = _mm
                        gx_last_mm = _mm
                    if (m + n0 // 512) % 2 == 0:
                        nc.scalar.activation(
                            Gx[:, m, n0 : n0 + nn], ps[:, :nn],
                            AF.Identity, bias=bias_sb[:, m : m + 1],
                        )
                    else:
                        nc.vector.tensor_scalar(
                            out=Gx[:, m, n0 : n0 + nn],
                            in0=ps[:, :nn],
                            scalar1=bias_sb[:, m : m + 1],
                            scalar2=None,
                            op0=ALU.add,
                        )

            # attention/vocab-phase loads (needed ~150us later); explicitly
            # held back behind the first Gx matmul so they don't steal DMA
            # bandwidth from the startup-critical loads
            import bass_rust as _br

            late = []
            late.append(nc.scalar.dma_start(enct_sb[:], d_enct[:].rearrange("k p n -> p k n")))
            late.append(nc.gpsimd.dma_start(oh_sb[:], d_oh[:]))
            late.append(nc.gpsimd.dma_start(mask_sb[:], d_mask[:]))
            late.append(nc.gpsimd.dma_start(awt_sb[:], d_awt[:].rearrange("k p m -> p k m")))
            late.append(nc.gpsimd.dma_start(attnb_sb[:], d_attnb[:]))
            late.append(nc.gpsimd.dma_start(pge_sb[:], d_pge[:]))
            late.append(nc.gpsimd.dma_start(pgd_sb[:], d_pgd[:]))
            late.append(nc.gpsimd.dma_start(pb_sb[:], d_pb[:]))
            late.append(nc.gpsimd.dma_start(conv_sb[:], d_conv[:]))
            encb_sb = pers.tile([L, B * EMB], dt.bfloat16, tag="encb")
            late.append(nc.gpsimd.dma_start(encb_sb[:], d_encb[:]))
            Wt_sb = pers.tile([128, 4, VL], dt.bfloat16, tag="Wt")
            late.append(nc.gpsimd.dma_start(Wt_sb[:], d_wt[:].rearrange("k p v -> p k v")))
            for dma in late:
                _br.add_dep_helper(
                    dma.ins, gx_last_mm.ins, True,
                    "hold noncritical DMA behind the Gx phase",
                )

            # ---------- LSTM over T steps ----------
            # HT column slots: slot 0 = h_{-1} = 0; step t writes slot t+1.
            nc.vector.memset(HT[:, :, 0:32], 0.0)

            # Gate tiles are host-permuted into two half-banks:
            #   bank A (m 0..7)  = [i0 i1 f0 f1 o0 o1 g0 g1]  -> h quarters 0,1
            #   bank B (m 8..15) = [i2 i3 f2 f3 o2 o3 g2 g3]  -> h quarters 2,3
            # Each bank gets its own PSUM tile, preloaded with Gx_t via an
            # eye-matmul two steps ahead (PE-idle time), so the tail of half A
            # can start after only the A-half of the W_hh matmuls.
            psl = [None] * (T + 2)

            def eye_preload(t):
                if t >= T:
                    return
                pa = psA.tile([128, 256], dt.float32, tag="psLa", bufs=2)
                pb = psA.tile([128, 256], dt.float32, tag="psLb", bufs=2)
                nc.tensor.matmul(
                    pa[:, :], eye_sb[:],
                    Gx[:, 0:8, t * 32 : (t + 1) * 32],
                    start=True, stop=True,
                )
                nc.tensor.matmul(
                    pb[:, :], eye_sb[:],
                    Gx[:, 8:16, t * 32 : (t + 1) * 32],
                    start=True, stop=True,
                )
                psl[t] = (pa, pb)

            eye_preload(0)

            cprev = None
            act_prev = None  # forced ACT-queue chain (scheduler reorders otherwise)

            def act_chain(ins):
                nonlocal act_prev
                if act_prev is not None:
                    _br.add_dep_helper(ins.ins, act_prev.ins, True, "ACT order")
                act_prev = ins
                return ins

            for t in range(T):
                ht_prev = HT[:, :, t * 32 : (t + 1) * 32]
                pa, pb = psl[t]
                # += W_hh @ h_{t-1}; A-half first so its tail starts earliest.
                for half, ps in ((0, pa), (1, pb)):
                    for k in range(4):
                        for m in range(8):
                            nc.tensor.matmul(
                                ps[:, m * 32 : (m + 1) * 32],
                                whh_sb[:, k, (8 * half + m) * 128 : (8 * half + m + 1) * 128],
                                ht_prev[:, k, :],
                                start=False,
                                stop=(k == 3),
                                skip_group_check=True,
                            )
                eye_preload(t + 1)
                cnew = small.tile([128, 4, 32], dt.float32, tag="c_t")
                sigs = []
                tgs = []
                # ACT emission order: sigA, tgA, sigB, tcA, tgB, tcB — sigB
                # fills the ACT gap while DVE computes c half A.
                for hx, ps in ((0, pa), (1, pb)):
                    # bank layout [i i f f o o g g] x 32 cols
                    sig = small.tile([128, 3, 2, 32], dt.float32, tag=f"sig{hx}")
                    act_chain(nc.scalar.activation(
                        sig[:],
                        ps[:, 0:192].rearrange("p (g u x) -> p g u x", g=3, u=2),
                        AF.Sigmoid,
                    ))
                    sigs.append(sig)
                    tg = small.tile([128, 2, 32], dt.float32, tag=f"tg{hx}")
                    tgs.append(tg)
                    if hx == 0:
                        act_chain(nc.scalar.activation(
                            tg[:], ps[:, 192:256].rearrange("p (u x) -> p u x", u=2),
                            AF.Tanh,
                        ))
                for hx in (0, 1):
                    sig, tg = sigs[hx], tgs[hx]
                    if hx == 1:
                        act_chain(nc.scalar.activation(
                            tg[:], pb[:, 192:256].rearrange("p (u x) -> p u x", u=2),
                            AF.Tanh,
                        ))
                    if cprev is not None:
                        b_t = small.tile([128, 2, 32], dt.float32, tag=f"b_t{hx}")
                        nc.vector.tensor_mul(
                            b_t[:], sig[:, 1], cprev[:, 2 * hx : 2 * hx + 2]
                        )
                    a_t = small.tile([128, 2, 32], dt.float32, tag=f"a_t{hx}")
                    nc.vector.tensor_mul(a_t[:], sig[:, 0], tg[:])
                    if cprev is None:
                        nc.vector.tensor_copy(cnew[:, 2 * hx : 2 * hx + 2], a_t[:])
                    else:
                        nc.vector.tensor_add(
                            cnew[:, 2 * hx : 2 * hx + 2], a_t[:], b_t[:]
                        )
                    tc_t = small.tile([128, 2, 32], dt.float32, tag=f"tc_t{hx}")
                    act_chain(nc.scalar.activation(
                        tc_t[:], cnew[:, 2 * hx : 2 * hx + 2], AF.Tanh
                    ))
                    # per-quarter h writes (exact-match deps: the next step's
                    # k-pass waits only on its own quarter)
                    for u in (0, 1):
                        nc.vector.tensor_mul(
                            HT[
                                :,
                                2 * hx + u : 2 * hx + u + 1,
                                (t + 1) * 32 : (t + 2) * 32,
                            ],
                            sig[:, 2, u : u + 1],
                            tc_t[:, u : u + 1],
                        )
                cprev = cnew

            HTv = HT[:, :, 32 : 32 + R]  # h_1..h_T columns, time-major

            # ---------- attention (chunked psum, batch-major) ----------
            # scratch reuses arena slots that die with the LSTM
            P1 = arena.tile([32, 3 * R], dt.float32, tag="slotA")
            P1t = arena.tile([32, 3 * R], dt.float32, tag="slotB")
            Vt = arena.tile([32, 2 * R], dt.float32, tag="slotC")
            Wt2 = arena.tile([32, 2 * R], dt.float32, tag="slotD")
            nc.vector.memset(Vt[:], 0.0)
            # Everything after the LSTM works in batch-major columns
            # (c = b*T + t); time-major consumers read via strided views.
            # Q^T [256, R] time-major (scores read per-batch slices of it)
            for m in range(2):
                for n0 in range(0, R, 512):
                    nn = min(512, R - n0)
                    qp = psA.tile([128, 512], dt.float32, tag="psA", bufs=4)
                    for k in range(4):
                        nc.tensor.matmul(
                            qp[:, :nn],
                            awt_sb[:, k, m * 128 : (m + 1) * 128],
                            HTv[:, k, :][:, n0 : n0 + nn],
                            start=(k == 0),
                            stop=(k == 3),
                        )
                    if m == 0:
                        nc.vector.tensor_scalar(
                            out=Qsb[:, m, n0 : n0 + nn], in0=qp[:, :nn],
                            scalar1=attnb_sb[:, m : m + 1], scalar2=None,
                            op0=ALU.add,
                        )
                    else:
                        nc.scalar.activation(
                            Qsb[:, m, n0 : n0 + nn], qp[:, :nn],
                            AF.Identity, bias=attnb_sb[:, m : m + 1],
                        )

            # scores/E chunks [64, 512] batch-major; exp+mask fused per chunk
            for n0 in range(0, R, 512):
                nn = min(512, R - n0)
                nb = nn // T
                b0 = n0 // T
                sct = psA.tile([128, 512], dt.float32, tag="psA", bufs=4)
                for bb in range(nb):
                    b = b0 + bb
                    for k in range(2):
                        nc.tensor.matmul(
                            sct[0:L, bb * T : (bb + 1) * T],
                            enct_sb[:, k, b * L : (b + 1) * L],
                            Qsb[:, k, :].rearrange("p (t bb) -> p bb t", bb=B)[:, b, :],
                            start=(k == 0),
                            stop=(k == 1),
                        )
                nc.scalar.activation(Ebf[:, n0 : n0 + nn], sct[0:L, :nn], AF.Exp)
                mb = mask_sb[:, :]
                nc.vector.tensor_mul(
                    Ebf[:, n0 : n0 + nn].rearrange("p (bb t) -> p bb t", t=T),
                    Ebf[:, n0 : n0 + nn].rearrange("p (bb t) -> p bb t", t=T),
                    bass.AP(tensor=mb.tensor, offset=mb.offset + b0,
                            ap=[list(mb.ap[0]), [1, nb], [0, T]]),
                )

            # ctx_un^T [256, R] bf16 batch-major
            for m in range(2):
                for n0 in range(0, R, 512):
                    nn = min(512, R - n0)
                    nb = nn // T
                    b0 = n0 // T
                    cp = psA.tile([128, 512], dt.float32, tag="psA", bufs=4)
                    for bb in range(nb):
                        b = b0 + bb
                        nc.tensor.matmul(
                            cp[:, bb * T : (bb + 1) * T],
                            encb_sb[:, b * EMB + m * 128 : b * EMB + (m + 1) * 128],
                            Ebf[:, b * T : (b + 1) * T],
                        )
                    if m == 0:
                        nc.scalar.copy(ctxsb[:, m, n0 : n0 + nn], cp[:, :nn])
                    else:
                        nc.vector.tensor_copy(ctxsb[:, m, n0 : n0 + nn], cp[:, :nn])

            # cs/pp/ph as 32-row-replicated psum blocks -> P1 [96, R] sbuf
            for n0 in range(0, R, 512):
                nn = min(512, R - n0)
                csp = psA.tile([128, 512], dt.float32, tag="psA", bufs=4)
                nc.tensor.matmul(csp[0:32, :nn], ones_b64[:], Ebf[:, n0 : n0 + nn])
                nc.scalar.copy(P1[:, n0 : n0 + nn], csp[0:32, :nn])
                ppp = psA.tile([128, 512], dt.float32, tag="psA", bufs=4)
                for k in range(2):
                    nc.tensor.matmul(
                        ppp[0:32, :nn], pge_sb[:, k], ctxsb[:, k, n0 : n0 + nn],
                        start=(k == 0), stop=(k == 1),
                    )
                nc.vector.tensor_copy(P1[:, R + n0 : R + n0 + nn], ppp[0:32, :nn])
                php = psA.tile([128, 512], dt.float32, tag="psA", bufs=4)
                for k in range(4):
                    nc.tensor.matmul(
                        php[0:32, :nn], pgd_sb[:, k], HTv[:, k, :][:, n0 : n0 + nn],
                        start=(k == 0), stop=(k == 3),
                    )
                nc.scalar.copy(P1[:, 2 * R + n0 : 2 * R + n0 + nn], php[0:32, :nn])

            # S_un^T [91, R] bf16 batch-major
            for n0 in range(0, R, 512):
                nn = min(512, R - n0)
                nb = nn // T
                b0 = n0 // T
                spt = psA.tile([128, 512], dt.float32, tag="psA", bufs=4)
                for bb in range(nb):
                    b = b0 + bb
                    nc.tensor.matmul(
                        spt[0:NOBJ, bb * T : (bb + 1) * T],
                        oh_sb[:, b * NOBJ : (b + 1) * NOBJ],
                        Ebf[:, b * T : (b + 1) * T],
                    )
                nb2 = nn // T
                b02 = n0 // T
                nc.vector.tensor_copy(
                    Ssu[:, 0:R].rearrange("p (t bb) -> p t bb", bb=B)[:, :, b02 : b02 + nb2],
                    spt[0:NOBJ, :nn].rearrange("p (bb t) -> p t bb", t=T),
                )

            # p_gen math on 32x32 stream-transposed data (no [1,R] lane ops)
            # chunked: each transpose starts as soon as its P1 range is copied
            for q3i in range(3):
                for tn0 in range(0, R, 512):
                    nc.vector.transpose(
                        P1t[:, q3i * R + tn0 : q3i * R + tn0 + 512],
                        P1[:, q3i * R + tn0 : q3i * R + tn0 + 512],
                    )
            csv = P1t[:, 0:R].rearrange("p (j f) -> p j f", f=32)[:, :, 0]
            ppv = P1t[:, R : 2 * R].rearrange("p (j f) -> p j f", f=32)[:, :, 0]
            phv = P1t[:, 2 * R : 3 * R].rearrange("p (j f) -> p j f", f=32)[:, :, 0]
            rv = small.tile([32, 32], dt.float32, tag="rv32")
            nc.vector.reciprocal(rv[:], csv)
            en32 = small.tile([32, 32], dt.float32, tag="en32")
            nc.vector.tensor_mul(en32[:], rv[:], ppv)
            # phv is time-major-folded; transpose its 32x32 to match (b=j,t=p)
            phc = small.tile([32, 32], dt.float32, tag="phc")
            nc.vector.tensor_copy(phc[:], phv)
            ph32 = small.tile([32, 32], dt.float32, tag="ph32")
            nc.vector.transpose(ph32[:], phc[:])
            den32 = small.tile([32, 32], dt.float32, tag="den32")
            nc.vector.tensor_add(den32[:], en32[:], ph32[:])
            p32 = small.tile([32, 32], dt.float32, tag="p32")
            nc.scalar.activation(p32[:], den32[:], AF.Sigmoid, bias=pb_sb[:, 0:1])
            q32 = small.tile([32, 32], dt.float32, tag="q32")
            nc.vector.tensor_scalar(
                out=q32[:], in0=p32[:], scalar1=-1.0, scalar2=1.0,
                op0=ALU.mult, op1=ALU.add,
            )
            s32 = small.tile([32, 32], dt.float32, tag="s32")
            nc.vector.tensor_mul(s32[:], rv[:], q32[:])
            p32T = small.tile([32, 32], dt.float32, tag="p32T")
            nc.vector.transpose(p32T[:], p32[:])
            s32T = small.tile([32, 32], dt.float32, tag="s32T")
            nc.vector.transpose(s32T[:], s32[:])
            nc.vector.tensor_copy(
                Vt[:, 0:R].rearrange("p (j f) -> p j f", f=32)[:, :, 0], p32T[:]
            )
            nc.vector.tensor_copy(
                Vt[:, R : 2 * R].rearrange("p (j f) -> p j f", f=32)[:, :, 0], s32T[:]
            )
            for q2i in range(2):
                for tn0 in range(0, R, 512):
                    nc.vector.transpose(
                        Wt2[:, q2i * R + tn0 : q2i * R + tn0 + 512],
                        Vt[:, q2i * R + tn0 : q2i * R + tn0 + 512],
                    )
            p_row = Wt2[0:1, 0:R]           # [1, R] TIME-major
            sscl_row = Wt2[0:1, R : 2 * R]  # [1, R] TIME-major

            # Hs = H * bcast(p)  (time-major bf16) ; Ss2 row 96 = p
            nc.vector.memset(Ss2[64:96, :], 0.0)
            for n0 in range(0, R, 512):
                nn = min(512, R - n0)
                pbt = psA.tile([128, 512], dt.float32, tag="psA", bufs=4)
                nc.tensor.matmul(pbt[:, :nn], ones_f[:], p_row[:, n0 : n0 + nn])
                for k in range(4):
                    nc.vector.tensor_mul(
                        Hs[:, k, n0 : n0 + nn], HTv[:, k, :][:, n0 : n0 + nn],
                        pbt[:, :nn],
                    )
                qbt = psA.tile([128, 512], dt.float32, tag="psA", bufs=4)
                nc.tensor.matmul(
                    qbt[0:NOBJ, :nn], ones_f[:, 0:NOBJ], sscl_row[:, n0 : n0 + nn]
                )
                nc.vector.tensor_mul(
                    Ss2[0:NOBJ, n0 : n0 + nn], Ssu[:, n0 : n0 + nn], qbt[0:NOBJ, :nn]
                )
            nc.scalar.copy(Ss2[96:97, :], p_row)

            if DBG:
                nc.sync.dma_start(d_ebf[:], Ebf[:])
                nc.sync.dma_start(d_p1[:], P1[:])
                nc.sync.dma_start(d_p1t[:], P1t[:])
                nc.sync.dma_start(d_wt2[:], Wt2[:])
                nc.sync.dma_start(d_ss2[:], Ss2[:])
                nc.sync.dma_start(d_hs[:], Hs[:])
                nc.sync.dma_start(d_ht[:], HTv[:, :, :])

            # ---------- vocab matmul, vocab-sharded ----------
            # lhsT reads Hs/Ss2 (batch-major) through time-major strided views
            for m in range(MT):
                rlo = m * 128
                for n0 in range(0, VL, 512):
                    nn = min(512, VL - n0)
                    ps = psA.tile([128, 512], dt.float32, tag="psA", bufs=4)
                    for k in range(4):
                        nc.tensor.matmul(
                            ps[:, :nn],
                            Hs[:, k, rlo : rlo + 128],
                            Wt_sb[:, k, n0 : n0 + nn],
                            start=(k == 0),
                            stop=False,
                        )
                    # pointer logits + p*linear_b in one K=97 matmul
                    nc.tensor.matmul(
                        ps[:, :nn],
                        Ss2[:, rlo : rlo + 128],
                        conv_sb[:, n0 : n0 + nn],
                        start=False,
                        stop=True,
                    )
                    om = omp.tile([128, 512], dt.bfloat16, tag="om")
                    if (m + n0 // 512) % 2 == 0:
                        nc.scalar.copy(om[:, :nn], ps[:, :nn])
                        nc.sync.dma_start(d_out[rlo : rlo + 128, n0 : n0 + nn], om[:, :nn])
                    else:
                        nc.vector.tensor_copy(om[:, :nn], ps[:, :nn])
                        nc.scalar.dma_start(d_out[rlo : rlo + 128, n0 : n0 + nn], om[:, :nn])

    n_fix = _split_multi_waits(nc, mybir, max_waits=1)
    nc.finalize()
    return nc


def _prep_inputs(features, captions, lengths, encoder_input, encoder_output,
                 embed_W, W_ih, W_hh, b_ih, b_hh, linear_W, linear_b,
                 attn_W, attn_b, pge_W, pge_b, pgd_W, pgd_b, converter):
    """Host-side sharding/layout prep. Returns per-core in_maps and T."""
    import ml_dtypes

    bf16 = ml_dtypes.bfloat16
    f32 = np.float32

    features = np.asarray(features, f32)
    captions = np.asarray(captions)
    encoder_input = np.asarray(encoder_input)
    encoder_output = np.asarray(encoder_output, f32)
    embed_W = np.asarray(embed_W, f32)
    W_ih = np.asarray(W_ih, f32)
    W_hh = np.asarray(W_hh, f32)
    b_ih = np.asarray(b_ih, f32)
    b_hh = np.asarray(b_hh, f32)
    linear_W = np.asarray(linear_W, f32)
    linear_b = np.asarray(linear_b, f32)
    attn_W = np.asarray(attn_W, f32)
    attn_b = np.asarray(attn_b, f32)
    pge_W = np.asarray(pge_W, f32)
    pge_b = np.asarray(pge_b, f32)
    pgd_W = np.asarray(pgd_W, f32)
    pgd_b = np.asarray(pgd_b, f32)
    converter = np.asarray(converter, f32)

    T = int(lengths)
    R = T * B

    # x sequence: t=0 -> features, t>=1 -> embed_W[captions[:, t-1]]
    emb = np.empty((B, T, EMB), f32)
    emb[:, 0, :] = features
    if T > 1:
        emb[:, 1:, :] = embed_W[captions[:, : T - 1]]
    # XT [EMB, R], column c = t*B + b
    XT = np.ascontiguousarray(emb.transpose(2, 1, 0).reshape(EMB, R))
    xt = XT.reshape(2, 128, R).astype(bf16)

    # gate permutation into two half-banks (torch chunk order: i=0-3, f=4-7,
    # g=8-11, o=12-15): A = [i0 i1 f0 f1 o0 o1 g0 g1], B = same second halves
    chunk_perm = [0, 1, 4, 5, 12, 13, 8, 9, 2, 3, 6, 7, 14, 15, 10, 11]
    perm = np.concatenate([np.arange(c * 128, (c + 1) * 128) for c in chunk_perm])
    wih = np.ascontiguousarray(W_ih[perm].T).reshape(2, 128, 4 * HID).astype(bf16)
    whh = np.ascontiguousarray(W_hh[perm].T).reshape(4, 128, 4 * HID).astype(bf16)
    biasv = (b_ih + b_hh)[perm].astype(f32)
    biaspm = np.ascontiguousarray(biasv.reshape(16, 128).T)

    encb = np.ascontiguousarray(
        encoder_output.transpose(1, 0, 2).reshape(L, B * EMB)
    ).astype(bf16)
    enct = np.ascontiguousarray(
        encoder_output.transpose(2, 0, 1).reshape(2, 128, B * L)
    ).astype(bf16)
    oh = np.ascontiguousarray(
        np.eye(NOBJ, dtype=f32)[encoder_input].transpose(1, 0, 2).reshape(L, B * NOBJ)
    ).astype(bf16)
    mask01 = np.ascontiguousarray((encoder_input.T != 0).astype(f32))  # [L, B]
    eye = np.eye(128, dtype=f32).astype(bf16)

    awt = np.ascontiguousarray(attn_W.T).reshape(4, 128, EMB).astype(bf16)
    attnb = np.ascontiguousarray(attn_b.reshape(2, 128).T).astype(f32)
    pge = np.repeat(
        np.ascontiguousarray(pge_W.reshape(EMB).reshape(2, 128).T)[:, :, None], 32, axis=2
    ).astype(bf16)
    pgd = np.repeat(
        np.ascontiguousarray(pgd_W.reshape(HID).reshape(4, 128).T)[:, :, None], 32, axis=2
    ).astype(bf16)
    pb = np.full((32, 1), float(pge_b.reshape(-1)[0] + pgd_b.reshape(-1)[0]), f32)

    common = dict(
        xt=xt, wih=wih, biaspm=biaspm, whh=whh, encb=encb, enct=enct,
        oh=oh, mask01=mask01, eye=eye, awt=awt, attnb=attnb, pge=pge,
        pgd=pgd, pb=pb,
    )

    in_maps = []
    for c in range(NCORES):
        v0, v1 = c * VL, (c + 1) * VL
        wt = np.ascontiguousarray(linear_W[v0:v1].T).reshape(4, 128, VL).astype(bf16)
        # converter slice with linear_b appended as row 91 (paired with the
        # p_gen row of Ss2 on device)
        conv = np.ascontiguousarray(
            np.concatenate(
                [converter[:, v0:v1], np.zeros((5, VL), f32),
                 linear_b[v0:v1][None, :]], axis=0)
        ).astype(bf16)
        m = dict(common)
        m.update(wt=wt, conv=conv)
        in_maps.append(m)
    return in_maps, T, R


def kernel(**inputs):
    global LAST_RESULT
    from concourse.bass_utils import run_bass_kernel_spmd

    in_maps, T, R = _prep_inputs(**inputs)
    if T not in _CACHE:
        _CACHE[T] = _build(T)
    nc = _CACHE[T]

    res = run_bass_kernel_spmd(nc, in_maps, core_ids=list(range(NCORES)))
    LAST_RESULT = res
    out = np.concatenate([res.results[c]["out"] for c in range(NCORES)], axis=1)
    return out.astype(np.float32)



# revision 51
# speedup vs baseline: 1.0267x; 1.0054x over previous
"""Trainium2 Bass kernel for nn_DecoderRNN (LSTM decoder + pointer-generator).

Strategy (8 NeuronCores, SPMD, no collectives):
  - LSTM + attention replicated on every core over the full batch (the LSTM
    recurrence cost is weight-bound, independent of batch, so sharding batch
    would not reduce it; replication avoids collectives entirely).
  - The 30000-wide output matmul is vocab-sharded: core c computes output
    columns [c*3750, (c+1)*3750) and the host concatenates.
  - Pointer path is rewritten: outputs_pointer = (aw @ onehot(enc_in)) @ converter
    so the 30000-wide gather of `converter[encoder_input]` never materializes.
  - p_gen combine is folded into the matmul inputs:
        out = (H*p) @ W.T + p x linear_b + ((1-p)*S) @ converter
  - Softmax normalization is applied to exp(scores) directly (no max-subtract;
    scores are O(25) so fp32 exp is safe).

Layouts are "transposed": rows of the output (time-major index c = t*B + b)
live on the free axis; hidden/gate/vocab dims live on partitions.
"""

import os

import numpy as np

B = 32
L = 64
EMB = 256
HID = 512
VOCAB = 30000
NOBJ = 91
NCORES = 8
VL = VOCAB // NCORES  # 3750

F32 = "float32"
BF16 = "bfloat16"

_CACHE = {}
LAST_RESULT = None


# ---------------------------------------------------------------------------
# walrus CTRL-encoding legalization: hoist extra sem-waits onto same-engine NOPs
def _split_multi_waits(nc, mybir, max_waits=1):
    n_fix = 0
    for f in nc.m.functions:
        for block in f.blocks:
            insts = list(block.instructions)
            out = []
            changed = False
            for inst in insts:
                si = inst.sync_info
                waits = list(si.on_wait) if si is not None else []
                if len(waits) > max_waits:
                    extra = waits[:-max_waits]
                    keep = waits[-max_waits:]
                    chunks = [
                        extra[i : i + max_waits]
                        for i in range(0, len(extra), max_waits)
                    ]
                    for ci, chunk in enumerate(chunks):
                        nop = mybir.InstNoOp(
                            name=f"{inst.name}-waitfix-{ci}",
                            engine=inst.engine,
                            sync_info=mybir.SyncInfo(on_wait=chunk, on_update=[]),
                            bass_nofuse=True,
                        )
                        nc.register_instruction(nop)
                        out.append(nop)
                    inst.sync_info = mybir.SyncInfo(
                        on_wait=keep, on_update=list(si.on_update)
                    )
                    n_fix += 1
                    changed = True
                out.append(inst)
            if changed:
                block.instructions = out
    return n_fix


# ---------------------------------------------------------------------------
def _build(T):
    """Build the SPMD Bass program for sequence length T. Returns (nc, meta)."""
    import concourse.bass as bass
    import concourse.tile as tile
    from concourse import mybir

    dt = mybir.dt
    AF = mybir.ActivationFunctionType
    ALU = mybir.AluOpType

    R = T * B  # number of output rows
    RP = ((R + 127) // 128) * 128  # padded to full partition tiles
    MT = RP // 128  # number of 128-row output tiles
    NCH = (VL + 511) // 512  # vocab N-chunks per core

    nc = bass.Bass()

    # ---------------- DRAM I/O ----------------
    d_xt = nc.dram_tensor("xt", [2, 128, R], dt.bfloat16, kind="ExternalInput")
    d_wih = nc.dram_tensor("wih", [2, 128, 4 * HID], dt.bfloat16, kind="ExternalInput")
    d_bias = nc.dram_tensor("biaspm", [128, 16], dt.float32, kind="ExternalInput")
    d_whh = nc.dram_tensor("whh", [4, 128, 4 * HID], dt.bfloat16, kind="ExternalInput")
    d_encb = nc.dram_tensor("encb", [L, B * EMB], dt.bfloat16, kind="ExternalInput")
    d_enct = nc.dram_tensor("enct", [2, 128, B * L], dt.bfloat16, kind="ExternalInput")
    d_oh = nc.dram_tensor("oh", [L, B * NOBJ], dt.bfloat16, kind="ExternalInput")
    d_mask = nc.dram_tensor("mask01", [L, B], dt.float32, kind="ExternalInput")
    d_eye = nc.dram_tensor("eye", [128, 128], dt.bfloat16, kind="ExternalInput")
    d_awt = nc.dram_tensor("awt", [4, 128, EMB], dt.bfloat16, kind="ExternalInput")
    d_attnb = nc.dram_tensor("attnb", [128, 2], dt.float32, kind="ExternalInput")
    d_pge = nc.dram_tensor("pge", [128, 2, 32], dt.bfloat16, kind="ExternalInput")
    d_pgd = nc.dram_tensor("pgd", [128, 4, 32], dt.bfloat16, kind="ExternalInput")
    d_pb = nc.dram_tensor("pb", [32, 1], dt.float32, kind="ExternalInput")
    d_wt = nc.dram_tensor("wt", [4, 128, VL], dt.bfloat16, kind="ExternalInput")
    d_conv = nc.dram_tensor("conv", [NOBJ + 6, VL], dt.bfloat16, kind="ExternalInput")
    d_out = nc.dram_tensor("out", [R, VL], dt.bfloat16, kind="ExternalOutput")
    DBG = bool(int(os.environ.get("KDBG", "0")))
    if DBG:
        d_ebf = nc.dram_tensor("dbg_ebf", [L, R], dt.bfloat16, kind="ExternalOutput")
        d_p1 = nc.dram_tensor("dbg_p1", [32, 3 * R], dt.float32, kind="ExternalOutput")
        d_p1t = nc.dram_tensor("dbg_p1t", [32, 3 * R], dt.float32, kind="ExternalOutput")
        d_wt2 = nc.dram_tensor("dbg_wt2", [32, 2 * R], dt.float32, kind="ExternalOutput")
        d_ss2 = nc.dram_tensor("dbg_ss2", [NOBJ + 6, RP], dt.bfloat16, kind="ExternalOutput")
        d_hs = nc.dram_tensor("dbg_hs", [128, 4, RP], dt.bfloat16, kind="ExternalOutput")
        d_ht = nc.dram_tensor("dbg_ht", [128, 4, R], dt.bfloat16, kind="ExternalOutput")

    with tile.TileContext(nc) as tc:
        with (
            tc.tile_pool(name="pers", bufs=1) as pers,
            tc.tile_pool(name="arena", bufs=1) as arena,
            tc.tile_pool(name="small", bufs=2) as small,
            tc.tile_pool(name="omp", bufs=4) as omp,
            tc.tile_pool(name="psA", bufs=3, space="PSUM") as psA,
            tc.tile_pool(name="psB", bufs=1, space="PSUM") as psB,
        ):
            # ---------- persistent tiles ----------
            HT = pers.tile([128, 4, 32 * (T + 1)], dt.bfloat16, tag="HT")
            Hs = pers.tile([128, 4, RP], dt.bfloat16, tag="Hs")
            Ss2 = pers.tile([NOBJ + 6, RP], dt.bfloat16, tag="Ss")
            Ssu = pers.tile([NOBJ, RP], dt.bfloat16, tag="Ssu")
            Qsb = pers.tile([128, 2, R], dt.bfloat16, tag="Qsb")
            Ebf = pers.tile([L, R], dt.bfloat16, tag="Ebf")
            ctxsb = pers.tile([128, 2, R], dt.bfloat16, tag="ctxsb")
            enct_sb = pers.tile([128, 2, B * L], dt.bfloat16, tag="enct")
            oh_sb = pers.tile([L, B * NOBJ], dt.bfloat16, tag="oh")
            mask_sb = pers.tile([L, B], dt.float32, tag="mask")
            eye_sb = pers.tile([128, 128], dt.bfloat16, tag="eye")
            awt_sb = pers.tile([128, 4, EMB], dt.bfloat16, tag="awt")
            attnb_sb = pers.tile([128, 2], dt.float32, tag="attnb")
            pge_sb = pers.tile([128, 2, 32], dt.bfloat16, tag="pge")
            pgd_sb = pers.tile([128, 4, 32], dt.bfloat16, tag="pgd")
            pb_sb = pers.tile([32, 1], dt.float32, tag="pb")
            bias_sb = pers.tile([128, 16], dt.float32, tag="biaspm")
            conv_sb = pers.tile([NOBJ + 6, VL], dt.bfloat16, tag="conv")
            ones_f = pers.tile([1, 128], dt.float32, tag="ones_f")
            ones64 = pers.tile([L, 1], dt.bfloat16, tag="ones64")
            ones_b64 = pers.tile([L, 32], dt.bfloat16, tag="ones_b64")
            ones_bf = pers.tile([64, 128], dt.bfloat16, tag="ones_bf")

            nc.vector.memset(ones_f[:], 1.0)
            nc.vector.memset(ones64[:], 1.0)
            nc.vector.memset(ones_bf[:], 1.0)
            nc.vector.memset(ones_b64[:], 1.0)

            # ---------- phase-0 DMAs (sync/HWDGE) ----------
            xt_sb = arena.tile([128, 2, R], dt.bfloat16, tag="slotD")
            wih_sb = arena.tile([128, 2, 4 * HID], dt.bfloat16, tag="slotC")
            Gx = arena.tile([128, 16, R], dt.bfloat16, tag="slotA")
            whh_sb = arena.tile([128, 4, 4 * HID], dt.bfloat16, tag="slotB")

            # spread input loads over three DMA queues so the Gx inputs
            # (sync queue) are not stuck behind the big attention loads
            # critical-path loads only (Gx then LSTM); everything needed for
            # the attention/vocab phases is issued after the Gx matmuls so it
            # doesn't steal DMA bandwidth from the startup
            nc.sync.dma_start(xt_sb[:, 0], d_xt[0])
            nc.scalar.dma_start(xt_sb[:, 1], d_xt[1])
            nc.sync.dma_start(wih_sb[:, 0], d_wih[0])
            nc.scalar.dma_start(wih_sb[:, 1], d_wih[1])
            nc.sync.dma_start(bias_sb[:], d_bias[:])
            nc.sync.dma_start(eye_sb[:], d_eye[:])
            nc.scalar.dma_start(whh_sb[:], d_whh[:].rearrange("k p m -> p k m"))

            # ---------- Gx = W_ih' @ X^T  (+bias on copy-out), bf16 ----------
            # Gx layout [128, 16 gate-tiles, R]; column c = t*B + b.
            gx_gate_mm = None
            for n0 in range(0, R, 512):
                nn = min(512, R - n0)
                for m in range(16):
                    ps = psA.tile([128, 512], dt.float32, tag="psA", bufs=4)
                    for k in range(2):
                        _mm = nc.tensor.matmul(
                            ps[:, :nn],
                            wih_sb[:, k, m * 128 : (m + 1) * 128],
                            xt_sb[:, k, n0 : n0 + nn],
                            start=(k == 0),
                            stop=(k == 1),
                        )
                        if gx_gate_mm is None:
                            gx_gate_mm = _mm
                        gx_last_mm = _mm
                    if (m + n0 // 512) % 2 == 0:
                        nc.scalar.activation(
                            Gx[:, m, n0 : n0 + nn], ps[:, :nn],
                            AF.Identity, bias=bias_sb[:, m : m + 1],
                        )
                    else:
                        nc.vector.tensor_scalar(
                            out=Gx[:, m, n0 : n0 + nn],
                            in0=ps[:, :nn],
                            scalar1=bias_sb[:, m : m + 1],
                            scalar2=None,
                            op0=ALU.add,
                        )

            # attention/vocab-phase loads (needed ~150us later); explicitly
            # held back behind the first Gx matmul so they don't steal DMA
            # bandwidth from the startup-critical loads
            import bass_rust as _br

            late = []
            late.append(nc.scalar.dma_start(enct_sb[:], d_enct[:].rearrange("k p n -> p k n")))
            late.append(nc.gpsimd.dma_start(oh_sb[:], d_oh[:]))
            late.append(nc.gpsimd.dma_start(mask_sb[:], d_mask[:]))
            late.append(nc.gpsimd.dma_start(awt_sb[:], d_awt[:].rearrange("k p m -> p k m")))
            late.append(nc.gpsimd.dma_start(attnb_sb[:], d_attnb[:]))
            late.append(nc.gpsimd.dma_start(pge_sb[:], d_pge[:]))
            late.append(nc.gpsimd.dma_start(pgd_sb[:], d_pgd[:]))
            late.append(nc.gpsimd.dma_start(pb_sb[:], d_pb[:]))
            late.append(nc.gpsimd.dma_start(conv_sb[:], d_conv[:]))
            encb_sb = pers.tile([L, B * EMB], dt.bfloat16, tag="encb")
            late.append(nc.gpsimd.dma_start(encb_sb[:], d_encb[:]))
            Wt_sb = pers.tile([128, 4, VL], dt.bfloat16, tag="Wt")
            late.append(nc.gpsimd.dma_start(Wt_sb[:], d_wt[:].rearrange("k p v -> p k v")))
            for dma in late:
                _br.add_dep_helper(
                    dma.ins, gx_last_mm.ins, True,
                    "hold noncritical DMA behind the Gx phase",
                )

            # ---------- LSTM over T steps ----------
            # HT column slots: slot 0 = h_{-1} = 0; step t writes slot t+1.
            nc.vector.memset(HT[:, :, 0:32], 0.0)

            # Gate tiles are host-permuted into two half-banks:
            #   bank A (m 0..7)  = [i0 i1 f0 f1 o0 o1 g0 g1]  -> h quarters 0,1
            #   bank B (m 8..15) = [i2 i3 f2 f3 o2 o3 g2 g3]  -> h quarters 2,3
            # Each bank gets its own PSUM tile, preloaded with Gx_t via an
            # eye-matmul two steps ahead (PE-idle time), so the tail of half A
            # can start after only the A-half of the W_hh matmuls.
            psl = [None] * (T + 2)

            def eye_preload(t):
                if t >= T:
                    return
                pa = psA.tile([128, 256], dt.float32, tag="psLa", bufs=2)
                pb = psA.tile([128, 256], dt.float32, tag="psLb", bufs=2)
                nc.tensor.matmul(
                    pa[:, :], eye_sb[:],
                    Gx[:, 0:8, t * 32 : (t + 1) * 32],
                    start=True, stop=True,
                )
                nc.tensor.matmul(
                    pb[:, :], eye_sb[:],
                    Gx[:, 8:16, t * 32 : (t + 1) * 32],
                    start=True, stop=True,
                )
                psl[t] = (pa, pb)

            eye_preload(0)

            cprev = None
            act_prev = None  # forced ACT-queue chain (scheduler reorders otherwise)

            def act_chain(ins):
                nonlocal act_prev
                if act_prev is not None:
                    _br.add_dep_helper(ins.ins, act_prev.ins, True, "ACT order")
                act_prev = ins
                return ins

            for t in range(T):
                ht_prev = HT[:, :, t * 32 : (t + 1) * 32]
                pa, pb = psl[t]
                # += W_hh @ h_{t-1}; A-half first so its tail starts earliest.
                for half, ps in ((0, pa), (1, pb)):
                    for k in range(4):
                        for m in range(8):
                            nc.tensor.matmul(
                                ps[:, m * 32 : (m + 1) * 32],
                                whh_sb[:, k, (8 * half + m) * 128 : (8 * half + m + 1) * 128],
                                ht_prev[:, k, :],
                                start=False,
                                stop=(k == 3),
                                skip_group_check=True,
                            )
                eye_preload(t + 1)
                cnew = small.tile([128, 4, 32], dt.float32, tag="c_t")
                sigs = []
                tgs = []
                # ACT emission order: sigA, tgA, sigB, tcA, tgB, tcB — sigB
                # fills the ACT gap while DVE computes c half A.
                for hx, ps in ((0, pa), (1, pb)):
                    # bank layout [i i f f o o g g] x 32 cols
                    sig = small.tile([128, 3, 2, 32], dt.float32, tag=f"sig{hx}")
                    act_chain(nc.scalar.activation(
                        sig[:],
                        ps[:, 0:192].rearrange("p (g u x) -> p g u x", g=3, u=2),
                        AF.Sigmoid,
                    ))
                    sigs.append(sig)
                    tg = small.tile([128, 2, 32], dt.float32, tag=f"tg{hx}")
                    tgs.append(tg)
                    if hx == 0:
                        act_chain(nc.scalar.activation(
                            tg[:], ps[:, 192:256].rearrange("p (u x) -> p u x", u=2),
                            AF.Tanh,
                        ))
                for hx in (0, 1):
                    sig, tg = sigs[hx], tgs[hx]
                    if hx == 1:
                        act_chain(nc.scalar.activation(
                            tg[:], pb[:, 192:256].rearrange("p (u x) -> p u x", u=2),
                            AF.Tanh,
                        ))
                    if cprev is not None:
                        b_t = small.tile([128, 2, 32], dt.float32, tag=f"b_t{hx}")
                        nc.vector.tensor_mul(
                            b_t[:], sig[:, 1], cprev[:, 2 * hx : 2 * hx + 2]
                        )
                    a_t = small.tile([128, 2, 32], dt.float32, tag=f"a_t{hx}")
                    nc.vector.tensor_mul(a_t[:], sig[:, 0], tg[:])
                    if cprev is None:
                        nc.vector.tensor_copy(cnew[:, 2 * hx : 2 * hx + 2], a_t[:])
                    else:
                        nc.vector.tensor_add(
                            cnew[:, 2 * hx : 2 * hx + 2], a_t[:], b_t[:]
                        )
                    tc_t = small.tile([128, 2, 32], dt.float32, tag=f"tc_t{hx}")
                    act_chain(nc.scalar.activation(
                        tc_t[:], cnew[:, 2 * hx : 2 * hx + 2], AF.Tanh
                    ))
                    # per-quarter h writes (exact-match deps: the next step's
                    # k-pass waits only on its own quarter)
                    for u in (0, 1):
                        nc.vector.tensor_mul(
                            HT[
                                :,
                                2 * hx + u : 2 * hx + u + 1,
                                (t + 1) * 32 : (t + 2) * 32,
                            ],
                            sig[:, 2, u : u + 1],
                            tc_t[:, u : u + 1],
                        )
                cprev = cnew

            HTv = HT[:, :, 32 : 32 + R]  # h_1..h_T columns, time-major

            # ---------- attention (chunked psum, batch-major) ----------
            # scratch reuses arena slots that die with the LSTM
            P1 = arena.tile([32, 3 * R], dt.float32, tag="slotA")
            P1t = arena.tile([32, 3 * R], dt.float32, tag="slotB")
            Vt = arena.tile([32, 2 * R], dt.float32, tag="slotC")
            Wt2 = arena.tile([32, 2 * R], dt.float32, tag="slotD")
            nc.vector.memset(Vt[:], 0.0)
            # Everything after the LSTM works in batch-major columns
            # (c = b*T + t); time-major consumers read via strided views.
            # Q^T [256, R] time-major (scores read per-batch slices of it)
            for m in range(2):
                for n0 in range(0, R, 512):
                    nn = min(512, R - n0)
                    qp = psA.tile([128, 512], dt.float32, tag="psA", bufs=4)
                    for k in range(4):
                        nc.tensor.matmul(
                            qp[:, :nn],
                            awt_sb[:, k, m * 128 : (m + 1) * 128],
                            HTv[:, k, :][:, n0 : n0 + nn],
                            start=(k == 0),
                            stop=(k == 3),
                        )
                    if m == 0:
                        nc.vector.tensor_scalar(
                            out=Qsb[:, m, n0 : n0 + nn], in0=qp[:, :nn],
                            scalar1=attnb_sb[:, m : m + 1], scalar2=None,
                            op0=ALU.add,
                        )
                    else:
                        nc.scalar.activation(
                            Qsb[:, m, n0 : n0 + nn], qp[:, :nn],
                            AF.Identity, bias=attnb_sb[:, m : m + 1],
                        )

            # scores/E chunks [64, 512] batch-major; exp+mask fused per chunk
            for n0 in range(0, R, 512):
                nn = min(512, R - n0)
                nb = nn // T
                b0 = n0 // T
                sct = psA.tile([128, 512], dt.float32, tag="psA", bufs=4)
                for bb in range(nb):
                    b = b0 + bb
                    for k in range(2):
                        nc.tensor.matmul(
                            sct[0:L, bb * T : (bb + 1) * T],
                            enct_sb[:, k, b * L : (b + 1) * L],
                            Qsb[:, k, :].rearrange("p (t bb) -> p bb t", bb=B)[:, b, :],
                            start=(k == 0),
                            stop=(k == 1),
                        )
                nc.scalar.activation(Ebf[:, n0 : n0 + nn], sct[0:L, :nn], AF.Exp)
                mb = mask_sb[:, :]
                nc.vector.tensor_mul(
                    Ebf[:, n0 : n0 + nn].rearrange("p (bb t) -> p bb t", t=T),
                    Ebf[:, n0 : n0 + nn].rearrange("p (bb t) -> p bb t", t=T),
                    bass.AP(tensor=mb.tensor, offset=mb.offset + b0,
                            ap=[list(mb.ap[0]), [1, nb], [0, T]]),
                )

            # ctx_un^T [256, R] bf16 batch-major
            for m in range(2):
                for n0 in range(0, R, 512):
                    nn = min(512, R - n0)
                    nb = nn // T
                    b0 = n0 // T
                    cp = psA.tile([128, 512], dt.float32, tag="psA", bufs=4)
                    for bb in range(nb):
                        b = b0 + bb
                        nc.tensor.matmul(
                            cp[:, bb * T : (bb + 1) * T],
                            encb_sb[:, b * EMB + m * 128 : b * EMB + (m + 1) * 128],
                            Ebf[:, b * T : (b + 1) * T],
                        )
                    if m == 0:
                        nc.scalar.copy(ctxsb[:, m, n0 : n0 + nn], cp[:, :nn])
                    else:
                        nc.vector.tensor_copy(ctxsb[:, m, n0 : n0 + nn], cp[:, :nn])

            # cs/pp/ph as 32-row-replicated psum blocks -> P1 [96, R] sbuf
            for n0 in range(0, R, 512):
                nn = min(512, R - n0)
                csp = psA.tile([128, 512], dt.float32, tag="psA", bufs=4)
                nc.tensor.matmul(csp[0:32, :nn], ones_b64[:], Ebf[:, n0 : n0 + nn])
                nc.scalar.copy(P1[:, n0 : n0 + nn], csp[0:32, :nn])
                ppp = psA.tile([128, 512], dt.float32, tag="psA", bufs=4)
                for k in range(2):
                    nc.tensor.matmul(
                        ppp[0:32, :nn], pge_sb[:, k], ctxsb[:, k, n0 : n0 + nn],
                        start=(k == 0), stop=(k == 1),
                    )
                nc.vector.tensor_copy(P1[:, R + n0 : R + n0 + nn], ppp[0:32, :nn])
                php = psA.tile([128, 512], dt.float32, tag="psA", bufs=4)
                for k in range(4):
                    nc.tensor.matmul(
                        php[0:32, :nn], pgd_sb[:, k], HTv[:, k, :][:, n0 : n0 + nn],
                        start=(k == 0), stop=(k == 3),
                    )
                nc.scalar.copy(P1[:, 2 * R + n0 : 2 * R + n0 + nn], php[0:32, :nn])

            # S_un^T [91, R] bf16 batch-major
            for n0 in range(0, R, 512):
                nn = min(512, R - n0)
                nb = nn // T
                b0 = n0 // T
                spt = psA.tile([128, 512], dt.float32, tag="psA", bufs=4)
                for bb in range(nb):
                    b = b0 + bb
                    nc.tensor.matmul(
                        spt[0:NOBJ, bb * T : (bb + 1) * T],
                        oh_sb[:, b * NOBJ : (b + 1) * NOBJ],
                        Ebf[:, b * T : (b + 1) * T],
                    )
                nb2 = nn // T
                b02 = n0 // T
                nc.vector.tensor_copy(
                    Ssu[:, 0:R].rearrange("p (t bb) -> p t bb", bb=B)[:, :, b02 : b02 + nb2],
                    spt[0:NOBJ, :nn].rearrange("p (bb t) -> p t bb", t=T),
                )

            # p_gen math on 32x32 stream-transposed data (no [1,R] lane ops)
            # chunked: each transpose starts as soon as its P1 range is copied
            for q3i in range(3):
                for tn0 in range(0, R, 512):
                    nc.vector.transpose(
                        P1t[:, q3i * R + tn0 : q3i * R + tn0 + 512],
                        P1[:, q3i * R + tn0 : q3i * R + tn0 + 512],
                    )
            csv = P1t[:, 0:R].rearrange("p (j f) -> p j f", f=32)[:, :, 0]
            ppv = P1t[:, R : 2 * R].rearrange("p (j f) -> p j f", f=32)[:, :, 0]
            phv = P1t[:, 2 * R : 3 * R].rearrange("p (j f) -> p j f", f=32)[:, :, 0]
            rv = small.tile([32, 32], dt.float32, tag="rv32")
            nc.vector.reciprocal(rv[:], csv)
            en32 = small.tile([32, 32], dt.float32, tag="en32")
            nc.vector.tensor_mul(en32[:], rv[:], ppv)
            # phv is time-major-folded; transpose its 32x32 to match (b=j,t=p)
            phc = small.tile([32, 32], dt.float32, tag="phc")
            nc.vector.tensor_copy(phc[:], phv)
            ph32 = small.tile([32, 32], dt.float32, tag="ph32")
            nc.vector.transpose(ph32[:], phc[:])
            den32 = small.tile([32, 32], dt.float32, tag="den32")
            nc.vector.tensor_add(den32[:], en32[:], ph32[:])
            p32 = small.tile([32, 32], dt.float32, tag="p32")
            nc.scalar.activation(p32[:], den32[:], AF.Sigmoid, bias=pb_sb[:, 0:1])
            q32 = small.tile([32, 32], dt.float32, tag="q32")
            nc.vector.tensor_scalar(
                out=q32[:], in0=p32[:], scalar1=-1.0, scalar2=1.0,
                op0=ALU.mult, op1=ALU.add,
            )
            s32 = small.tile([32, 32], dt.float32, tag="s32")
            nc.vector.tensor_mul(s32[:], rv[:], q32[:])
            p32T = small.tile([32, 32], dt.float32, tag="p32T")
            nc.vector.transpose(p32T[:], p32[:])
            s32T = small.tile([32, 32], dt.float32, tag="s32T")
            nc.vector.transpose(s32T[:], s32[:])
            nc.vector.tensor_copy(
                Vt[:, 0:R].rearrange("p (j f) -> p j f", f=32)[:, :, 0], p32T[:]
            )
            nc.vector.tensor_copy(
                Vt[:, R : 2 * R].rearrange("p (j f) -> p j f", f=32)[:, :, 0], s32T[:]
            )
            for q2i in range(2):
                for tn0 in range(0, R, 512):
                    nc.vector.transpose(
                        Wt2[:, q2i * R + tn0 : q2i * R + tn0 + 512],
                        Vt[:, q2i * R + tn0 : q2i * R + tn0 + 512],
                    )
            p_row = Wt2[0:1, 0:R]           # [1, R] TIME-major
            sscl_row = Wt2[0:1, R : 2 * R]  # [1, R] TIME-major

            # Hs = H * bcast(p)  (time-major bf16) ; Ss2 row 96 = p
            nc.vector.memset(Ss2[64:96, :], 0.0)
            for n0 in range(0, R, 512):
                nn = min(512, R - n0)
                pbt = psA.tile([128, 512], dt.float32, tag="psA", bufs=4)
                nc.tensor.matmul(pbt[:, :nn], ones_f[:], p_row[:, n0 : n0 + nn])
                for k in range(4):
                    nc.vector.tensor_mul(
                        Hs[:, k, n0 : n0 + nn], HTv[:, k, :][:, n0 : n0 + nn],
                        pbt[:, :nn],
                    )
                qbt = psA.tile([128, 512], dt.float32, tag="psA", bufs=4)
                nc.tensor.matmul(
                    qbt[0:NOBJ, :nn], ones_f[:, 0:NOBJ], sscl_row[:, n0 : n0 + nn]
                )
                nc.vector.tensor_mul(
                    Ss2[0:NOBJ, n0 : n0 + nn], Ssu[:, n0 : n0 + nn], qbt[0:NOBJ, :nn]
                )
            for cn0 in range(0, R, 512):
                nc.scalar.copy(
                    Ss2[96:97, cn0 : cn0 + 512], p_row[:, cn0 : cn0 + 512]
                )

            if DBG:
                nc.sync.dma_start(d_ebf[:], Ebf[:])
                nc.sync.dma_start(d_p1[:], P1[:])
                nc.sync.dma_start(d_p1t[:], P1t[:])
                nc.sync.dma_start(d_wt2[:], Wt2[:])
                nc.sync.dma_start(d_ss2[:], Ss2[:])
                nc.sync.dma_start(d_hs[:], Hs[:])
                nc.sync.dma_start(d_ht[:], HTv[:, :, :])

            # ---------- vocab matmul, vocab-sharded ----------
            # lhsT reads Hs/Ss2 (batch-major) through time-major strided views
            for m in range(MT):
                rlo = m * 128
                for n0 in range(0, VL, 512):
                    nn = min(512, VL - n0)
                    ps = psA.tile([128, 512], dt.float32, tag="psA", bufs=4)
                    for k in range(4):
                        nc.tensor.matmul(
                            ps[:, :nn],
                            Hs[:, k, rlo : rlo + 128],
                            Wt_sb[:, k, n0 : n0 + nn],
                            start=(k == 0),
                            stop=False,
                        )
                    # pointer logits + p*linear_b in one K=97 matmul
                    nc.tensor.matmul(
                        ps[:, :nn],
                        Ss2[:, rlo : rlo + 128],
                        conv_sb[:, n0 : n0 + nn],
                        start=False,
                        stop=True,
                    )
                    om = omp.tile([128, 512], dt.bfloat16, tag="om")
                    if (m + n0 // 512) % 2 == 0:
                        nc.scalar.copy(om[:, :nn], ps[:, :nn])
                        nc.sync.dma_start(d_out[rlo : rlo + 128, n0 : n0 + nn], om[:, :nn])
                    else:
                        nc.vector.tensor_copy(om[:, :nn], ps[:, :nn])
                        nc.scalar.dma_start(d_out[rlo : rlo + 128, n0 : n0 + nn], om[:, :nn])

    n_fix = _split_multi_waits(nc, mybir, max_waits=1)
    nc.finalize()
    return nc


def _prep_inputs(features, captions, lengths, encoder_input, encoder_output,
                 embed_W, W_ih, W_hh, b_ih, b_hh, linear_W, linear_b,
                 attn_W, attn_b, pge_W, pge_b, pgd_W, pgd_b, converter):
    """Host-side sharding/layout prep. Returns per-core in_maps and T."""
    import ml_dtypes

    bf16 = ml_dtypes.bfloat16
    f32 = np.float32

    features = np.asarray(features, f32)
    captions = np.asarray(captions)
    encoder_input = np.asarray(encoder_input)
    encoder_output = np.asarray(encoder_output, f32)
    embed_W = np.asarray(embed_W, f32)
    W_ih = np.asarray(W_ih, f32)
    W_hh = np.asarray(W_hh, f32)
    b_ih = np.asarray(b_ih, f32)
    b_hh = np.asarray(b_hh, f32)
    linear_W = np.asarray(linear_W, f32)
    linear_b = np.asarray(linear_b, f32)
    attn_W = np.asarray(attn_W, f32)
    attn_b = np.asarray(attn_b, f32)
    pge_W = np.asarray(pge_W, f32)
    pge_b = np.asarray(pge_b, f32)
    pgd_W = np.asarray(pgd_W, f32)
    pgd_b = np.asarray(pgd_b, f32)
    converter = np.asarray(converter, f32)

    T = int(lengths)
    R = T * B

    # x sequence: t=0 -> features, t>=1 -> embed_W[captions[:, t-1]]
    emb = np.empty((B, T, EMB), f32)
    emb[:, 0, :] = features
    if T > 1:
        emb[:, 1:, :] = embed_W[captions[:, : T - 1]]
    # XT [EMB, R], column c = t*B + b
    XT = np.ascontiguousarray(emb.transpose(2, 1, 0).reshape(EMB, R))
    xt = XT.reshape(2, 128, R).astype(bf16)

    # gate permutation into two half-banks (torch chunk order: i=0-3, f=4-7,
    # g=8-11, o=12-15): A = [i0 i1 f0 f1 o0 o1 g0 g1], B = same second halves
    chunk_perm = [0, 1, 4, 5, 12, 13, 8, 9, 2, 3, 6, 7, 14, 15, 10, 11]
    perm = np.concatenate([np.arange(c * 128, (c + 1) * 128) for c in chunk_perm])
    wih = np.ascontiguousarray(W_ih[perm].T).reshape(2, 128, 4 * HID).astype(bf16)
    whh = np.ascontiguousarray(W_hh[perm].T).reshape(4, 128, 4 * HID).astype(bf16)
    biasv = (b_ih + b_hh)[perm].astype(f32)
    biaspm = np.ascontiguousarray(biasv.reshape(16, 128).T)

    encb = np.ascontiguousarray(
        encoder_output.transpose(1, 0, 2).reshape(L, B * EMB)
    ).astype(bf16)
    enct = np.ascontiguousarray(
        encoder_output.transpose(2, 0, 1).reshape(2, 128, B * L)
    ).astype(bf16)
    oh = np.ascontiguousarray(
        np.eye(NOBJ, dtype=f32)[encoder_input].transpose(1, 0, 2).reshape(L, B * NOBJ)
    ).astype(bf16)
    mask01 = np.ascontiguousarray((encoder_input.T != 0).astype(f32))  # [L, B]
    eye = np.eye(128, dtype=f32).astype(bf16)

    awt = np.ascontiguousarray(attn_W.T).reshape(4, 128, EMB).astype(bf16)
    attnb = np.ascontiguousarray(attn_b.reshape(2, 128).T).astype(f32)
    pge = np.repeat(
        np.ascontiguousarray(pge_W.reshape(EMB).reshape(2, 128).T)[:, :, None], 32, axis=2
    ).astype(bf16)
    pgd = np.repeat(
        np.ascontiguousarray(pgd_W.reshape(HID).reshape(4, 128).T)[:, :, None], 32, axis=2
    ).astype(bf16)
    pb = np.full((32, 1), float(pge_b.reshape(-1)[0] + pgd_b.reshape(-1)[0]), f32)

    common = dict(
        xt=xt, wih=wih, biaspm=biaspm, whh=whh, encb=encb, enct=enct,
        oh=oh, mask01=mask01, eye=eye, awt=awt, attnb=attnb, pge=pge,
        pgd=pgd, pb=pb,
    )

    in_maps = []
    for c in range(NCORES):
        v0, v1 = c * VL, (c + 1) * VL
        wt = np.ascontiguousarray(linear_W[v0:v1].T).reshape(4, 128, VL).astype(bf16)
        # converter slice with linear_b appended as row 91 (paired with the
        # p_gen row of Ss2 on device)
        conv = np.ascontiguousarray(
            np.concatenate(
                [converter[:, v0:v1], np.zeros((5, VL), f32),
                 linear_b[v0:v1][None, :]], axis=0)
        ).astype(bf16)
        m = dict(common)
        m.update(wt=wt, conv=conv)
        in_maps.append(m)
    return in_maps, T, R


def kernel(**inputs):
    global LAST_RESULT
    from concourse.bass_utils import run_bass_kernel_spmd

    in_maps, T, R = _prep_inputs(**inputs)
    if T not in _CACHE:
        _CACHE[T] = _build(T)
    nc = _CACHE[T]

    res = run_bass_kernel_spmd(nc, in_maps, core_ids=list(range(NCORES)))
    LAST_RESULT = res
    out = np.concatenate([res.results[c]["out"] for c in range(NCORES)], axis=1)
    return out.astype(np.float32)

